# revision 11
# baseline (speedup 1.0000x reference)
"""Trainium2 Bass kernel for nn_GCN1PoolNorm: 3-layer GCN + shared BatchNorm +
global max pool + MLP head.

Self-contained: takes FULL inputs, returns FULL output [N_GRAPHS, N_CLASSES].

v3 design — FULLY REPLICATED, ZERO COLLECTIVES:
On this runner every collective costs ~27-38 ms (software-emulated NRT), so
any sharded design is collective-bound. Instead every core runs the whole
graph; the host takes core 0's output. No cross-core traffic at all.

Per layer (full graph, N=50000 nodes, E=850k edges incl. self loops):
- Node features h_tilde = act * dis live in a local DRAM table [N, 128] bf16
  (cols 0:64 = payload, 64:128 = zero pad -> 256B rows for SWDGE dma_gather).
- Host sorts edges by (dst window of 128, src half, src), pads each
  (window, half) group to 128-edge blocks. Per chunk of WC windows:
  2 dma_gathers (int16 idx limit -> src < 32768 and rest) fetch h_tilde[src]
  rows edge-major; S one-hot blocks [128e, 128d] fp8 stream from DRAM.
- Aggregation per window: chained PE matmuls psum[128d, 64f] +=
  S_blk.T @ msgs_blk[:, 0:64]  (node-major).
- Downstream per 4-window group: U = ACT(psum) * dis (per-partition scale),
  PE transpose to feat-major, Z = W.T @ U, BN stats via ACT accum_out
  (locally -> global stats without any AllReduce), z spilled to DRAM.
- Act phase per group: z -> BN affine + relu -> (layers 1,2) transpose back,
  * dis, write table rows; (layer 3) pool graph segments on the fly.
- Head computed for all 64 graphs on every core.
"""
import numpy as np
import ml_dtypes

from concourse import bacc, mybir, tile
from concourse.bass_utils import run_bass_kernel_spmd
from concourse.masks import make_identity

f32 = mybir.dt.float32
bf16 = mybir.dt.bfloat16
fp8 = mybir.dt.float8e4
i16 = mybir.dt.int16

N_CORES = 8
P = 128          # partition / block / dst-window quantum
D = 64           # feature dim
HALF = 32768     # int16 gather index limit
TROW = 128       # table row width (64 feats + 64 zero pad) -> 256B rows
BN_EPS = 1e-5
SW = 64          # dst-window width (S block columns)
WC = 16          # dst windows per stream chunk
GW = 512 // SW   # windows per downstream group (512 nodes)


# ---------------------------------------------------------------- host prep

def _prep(x, edge_index, batch, n_classes):
    n_nodes = x.shape[0]
    n_graphs = int(batch.max()) + 1
    ntile = (n_nodes + SW - 1) // SW            # dst windows
    tsz = [min(SW, n_nodes - t * SW) for t in range(ntile)]

    src = edge_index[0].astype(np.int64)
    dst = edge_index[1].astype(np.int64)
    deg = np.bincount(dst, minlength=n_nodes).astype(np.int64)

    # edge stream (incl. self loops), sorted by (window, half, src)
    s = np.concatenate([src, np.arange(n_nodes)])
    dl = np.concatenate([dst, np.arange(n_nodes)])
    t = dl // SW
    h = (s >= HALF).astype(np.int64)
    col = dl % SW
    order = np.lexsort((s, h, t))
    s, t, h, col = s[order], t[order], h[order], col[order]
    cnts = np.zeros((ntile, 2), np.int64)
    np.add.at(cnts, (t, h), 1)

    nb = (cnts + P - 1) // P                    # [ntile, 2] blocks per group
    NB0 = int(nb[:, 0].sum())
    NB1 = int(nb[:, 1].sum())
    NBLK = NB0 + NB1
    cum0 = np.concatenate([[0], np.cumsum(nb[:, 0])]).astype(int)
    cum1 = np.concatenate([[0], np.cumsum(nb[:, 1])]).astype(int)
    sb = np.concatenate([[0], np.cumsum(nb.sum(axis=1))]).astype(int)

    idx0 = np.zeros(NB0 * P, np.int16)
    idx1 = np.zeros(NB1 * P, np.int16)
    S = np.zeros((P, NBLK, SW), ml_dtypes.float8_e4m3)
    off = 0
    for tt in range(ntile):
        for hh in (0, 1):
            n = int(cnts[tt, hh])
            e = slice(off, off + n)
            off += n
            q = np.arange(n)
            if hh == 0:
                idx0[cum0[tt] * P:cum0[tt] * P + n] = s[e]
                blk0 = sb[tt]
            else:
                idx1[cum1[tt] * P:cum1[tt] * P + n] = s[e] - HALF
                blk0 = sb[tt] + nb[tt, 0]
            S[q % P, blk0 + q // P, col[e]] = 1.0
    # wrap indices: idx g -> [g % 16, g // 16], replicated on 128 partitions
    idx0w = np.tile(idx0.reshape(-1, 16).T, (8, 1)).copy()
    idx1w = np.tile(idx1.reshape(-1, 16).T, (8, 1)).copy()

    # deg layouts (fp32)
    deg_pt = np.zeros((SW, ntile), np.float32)
    degf = deg.astype(np.float32)
    for tt in range(ntile):
        deg_pt[:tsz[tt], tt] = degf[tt * SW:tt * SW + tsz[tt]]

    # pooling segments grouped by 512-node groups (for on-the-fly pooling)
    gb = np.searchsorted(batch, np.arange(n_graphs + 1))
    ngrp = (ntile + GW - 1) // GW
    pool_segs = []                              # (grp, s0, s1, g) rel to group
    for gr in range(ngrp):
        a, b = gr * GW * SW, min((gr + 1) * GW * SW, n_nodes)
        for g in range(n_graphs):
            s0, e0 = max(a, int(gb[g])), min(b, int(gb[g + 1]))
            if s0 < e0:
                pool_segs.append((gr, s0 - a, e0 - a, g))

    cfg = dict(n_nodes=n_nodes, ntile=ntile, tsz=tsz,
               nb=nb.tolist(), NB0=NB0, NB1=NB1, NBLK=NBLK,
               cum0=cum0.tolist(), cum1=cum1.tolist(), sb=sb.tolist(),
               pool_segs=pool_segs, ngrp=ngrp,
               n_classes=n_classes, n_graphs=n_graphs)
    data = dict(idx0=idx0w, idx1=idx1w, S=S, deg_pt=deg_pt)
    return cfg, data


# ---------------------------------------------------------------- device build

def _build(cfg, reps=1, n_devices=N_CORES):
    ntile, tsz = cfg["ntile"], cfg["tsz"]
    nb, NB0, NB1, NBLK = cfg["nb"], cfg["NB0"], cfg["NB1"], cfg["NBLK"]
    cum0, cum1, sb = cfg["cum0"], cfg["cum1"], cfg["sb"]
    ncls = cfg["n_classes"]
    n_nodes, n_graphs = cfg["n_nodes"], cfg["n_graphs"]
    ngrp = cfg["ngrp"]
    ntp = ntile * SW
    RW = 512

    chunks = [(t0, min(t0 + WC, ntile)) for t0 in range(0, ntile, WC)]
    CB0 = max(cum0[t1] - cum0[t0] for t0, t1 in chunks)
    CB1 = max(cum1[t1] - cum1[t0] for t0, t1 in chunks)
    CBS = max(sb[t1] - sb[t0] for t0, t1 in chunks)

    nc = bacc.Bacc(trn_type="TRN2", target_bir_lowering=False, debug=False,
                   num_devices=n_devices)

    x_in = nc.dram_tensor("x", [n_nodes, D], f32, kind="ExternalInput").ap()
    S_in = nc.dram_tensor("S", [P, NBLK, SW], fp8, kind="ExternalInput").ap()
    idx0_in = nc.dram_tensor("idx0", [P, NB0 * 8], i16, kind="ExternalInput").ap()
    idx1_in = nc.dram_tensor("idx1", [P, NB1 * 8], i16, kind="ExternalInput").ap()
    deg_pt = nc.dram_tensor("deg_pt", [SW, ntile], f32,
                            kind="ExternalInput").ap()
    Ws = [nc.dram_tensor(f"W{i}", [D, D], bf16, kind="ExternalInput").ap()
          for i in (1, 2, 3)]
    gamma = nc.dram_tensor("gamma", [D, 1], f32, kind="ExternalInput").ap()
    beta = nc.dram_tensor("beta", [D, 1], f32, kind="ExternalInput").ap()
    lin1w = nc.dram_tensor("lin1w", [D, D], bf16, kind="ExternalInput").ap()
    lin1b = nc.dram_tensor("lin1b", [D, 1], f32, kind="ExternalInput").ap()
    lin2w = nc.dram_tensor("lin2w", [D, ncls], bf16, kind="ExternalInput").ap()
    lin2b = nc.dram_tensor("lin2b", [ncls, 1], f32, kind="ExternalInput").ap()
    out = nc.dram_tensor("out", [n_graphs, ncls], f32,
                         kind="ExternalOutput").ap()

    table = nc.dram_tensor("table", [ntp, TROW], bf16).ap()
    z_dram = nc.dram_tensor("z_dram", [D, ntp], bf16).ap()

    with tile.TileContext(nc) as tc:
        with (
            tc.tile_pool(name="const", bufs=1) as cpool,
            tc.tile_pool(name="sbuf_s", bufs=2) as spool,
            tc.tile_pool(name="msgs", bufs=2) as mpool,
            tc.tile_pool(name="idxp", bufs=2) as ipool,
            tc.tile_pool(name="work", bufs=3) as wpool,
            tc.tile_pool(name="ump", bufs=GW + 3) as umpool,
            tc.tile_pool(name="psacc", bufs=2, space="PSUM") as ps_acc,
            tc.tile_pool(name="pstru", bufs=2, space="PSUM") as ps_tru,
            tc.tile_pool(name="psz", bufs=2, space="PSUM") as ps_z,
        ):
            # ---- residents
            dis_pt = cpool.tile([SW, ntile], f32)
            sums = cpool.tile([D, ngrp], f32)
            sums2 = cpool.tile([D, ngrp], f32)
            W_sb = [cpool.tile([D, D], bf16, tag=f"W{i}", name=f"W{i}_sb")
                    for i in range(3)]
            for i in range(3):
                nc.sync.dma_start(out=W_sb[i][:], in_=Ws[i][:])
            gamma_sb = cpool.tile([D, 1], f32, tag="gamma")
            beta_sb = cpool.tile([D, 1], f32, tag="beta")
            nc.sync.dma_start(out=gamma_sb[:], in_=gamma[:])
            nc.sync.dma_start(out=beta_sb[:], in_=beta[:])
            l1w_sb = cpool.tile([D, D], bf16, tag="l1w")
            l1b_sb = cpool.tile([D, 1], f32, tag="l1b")
            l2w_sb = cpool.tile([D, ncls], bf16, tag="l2w")
            l2b_sb = cpool.tile([ncls, 1], f32, tag="l2b")
            nc.sync.dma_start(out=l1w_sb[:], in_=lin1w[:])
            nc.sync.dma_start(out=l1b_sb[:], in_=lin1b[:])
            nc.sync.dma_start(out=l2w_sb[:], in_=lin2w[:])
            nc.sync.dma_start(out=l2b_sb[:], in_=lin2b[:])
            identP = cpool.tile([P, P], bf16, tag="identP")
            make_identity(nc, identP[:])
            emb = cpool.tile([D, n_graphs], f32, tag="emb")
            eps_sb = cpool.tile([D, 1], f32, tag="eps")
            nc.gpsimd.memset(eps_sb[:], BN_EPS)

            # one-time zero of table right half + pad rows
            zpad = cpool.tile([P, TROW], bf16, tag="zpad")
            nc.gpsimd.memset(zpad[:], 0.0)
            for o in range(0, ntp, P):
                w = min(P, n_nodes - o)
                if w > 0:
                    nc.sync.dma_start(out=table[o:o + w, D:TROW],
                                      in_=zpad[:w, :D])
                if w < P:
                    w0 = max(w, 0)
                    nc.sync.dma_start(out=table[o + w0:o + P, :],
                                      in_=zpad[:P - w0, :])

            # ---- dis (node-major per-partition layout only)
            dptf = wpool.tile([SW, ntile], f32, tag="dptf")
            nc.sync.dma_start(out=dptf[:], in_=deg_pt[:])
            nc.scalar.activation(dis_pt[:], dptf[:],
                                 mybir.ActivationFunctionType.Sqrt, bias=1.0)
            nc.vector.reciprocal(dis_pt[:], dis_pt[:])

            # ---- table0 = bf16(x * dis)
            for tt in range(ntile):
                w = tsz[tt]
                xt = wpool.tile([SW, D], f32, tag="xt")
                nc.sync.dma_start(out=xt[:w, :],
                                  in_=x_in[tt * SW:tt * SW + w, :])
                xb = wpool.tile([SW, D], bf16, tag="xb")
                nc.scalar.activation(xb[:w, :], xt[:w, :],
                                     mybir.ActivationFunctionType.Copy,
                                     scale=dis_pt[:w, tt:tt + 1])
                nc.sync.dma_start(out=table[tt * SW:tt * SW + w, 0:D],
                                  in_=xb[:w, :])

            # ---- layers
            for rep in range(reps):
                for li in range(3):
                    last = (li == 2)
                    Wl = W_sb[li]
                    # aggregation + z, chunked
                    pend = []          # per-window node-major U psum tiles
                    gdone = 0

                    def flush_group(pend_tiles, gidx):
                        # pend_tiles: list of (t, w, um_tile)
                        ptru = ps_tru.tile([D, RW], bf16, tag="tru",
                                           space="PSUM")
                        gw = 0
                        for (tt_, w_, um_) in pend_tiles:
                            nc.tensor.transpose(ptru[:, gw:gw + w_],
                                                um_[:w_, :], identP[:w_, :w_])
                            gw += w_
                        ut = wpool.tile([D, RW], bf16, tag="ut")
                        nc.vector.tensor_copy(ut[:, :gw], ptru[:, :gw])
                        psz = ps_z.tile([D, RW], f32, tag="zt", space="PSUM")
                        nc.tensor.matmul(psz[:, :gw], lhsT=Wl[:],
                                         rhs=ut[:, :gw],
                                         start=True, stop=True)
                        zt = wpool.tile([D, RW], bf16, tag="ztile")
                        nc.scalar.activation(
                            zt[:, :gw], psz[:, :gw],
                            mybir.ActivationFunctionType.Copy,
                            accum_out=sums[:, gidx:gidx + 1])
                        sq = wpool.tile([D, RW], f32, tag="sq")
                        nc.scalar.activation(
                            sq[:, :gw], psz[:, :gw],
                            mybir.ActivationFunctionType.Square,
                            accum_out=sums2[:, gidx:gidx + 1])
                        g0 = pend_tiles[0][0] * SW
                        nc.sync.dma_start(out=z_dram[:, g0:g0 + gw],
                                          in_=zt[:, :gw])

                    for (t0, t1) in chunks:
                        nb0c = cum0[t1] - cum0[t0]
                        nb1c = cum1[t1] - cum1[t0]
                        nbsc = sb[t1] - sb[t0]
                        S_t = spool.tile([P, CBS, SW], fp8, tag="S")
                        nc.sync.dma_start(out=S_t[:, :nbsc, :],
                                          in_=S_in[:, sb[t0]:sb[t1], :])
                        m0 = mpool.tile([P, CB0, TROW], bf16, tag="m0")
                        if nb0c:
                            ix0 = ipool.tile([P, CB0 * 8], i16, tag="ix0")
                            nc.sync.dma_start(
                                out=ix0[:, :nb0c * 8],
                                in_=idx0_in[:, cum0[t0] * 8:cum0[t1] * 8])
                            nc.gpsimd.dma_gather(
                                m0[:, :nb0c, :], table[:HALF, :],
                                ix0[:, :nb0c * 8],
                                nb0c * P, nb0c * P, TROW,
                                single_packet=False)
                        m1 = mpool.tile([P, CB1, TROW], bf16, tag="m1")
                        if nb1c:
                            ix1 = ipool.tile([P, CB1 * 8], i16, tag="ix1")
                            nc.sync.dma_start(
                                out=ix1[:, :nb1c * 8],
                                in_=idx1_in[:, cum1[t0] * 8:cum1[t1] * 8])
                            nc.gpsimd.dma_gather(
                                m1[:, :nb1c, :], table[HALF:ntp, :],
                                ix1[:, :nb1c * 8],
                                nb1c * P, nb1c * P, TROW,
                                single_packet=False)
                        for tt in range(t0, t1):
                            w = tsz[tt]
                            nblocks = nb[tt][0] + nb[tt][1]
                            ps = ps_acc.tile([SW, D], f32, tag="acc",
                                             space="PSUM")
                            for i in range(nblocks):
                                if i < nb[tt][0]:
                                    rhs = m0[:, cum0[tt] - cum0[t0] + i, 0:D]
                                else:
                                    rhs = m1[:, cum1[tt] - cum1[t0]
                                             + (i - nb[tt][0]), 0:D]
                                lhs = S_t[:, sb[tt] - sb[t0] + i, :]
                                nc.tensor.matmul(
                                    ps[:], lhsT=lhs, rhs=rhs,
                                    start=(i == 0), stop=(i == nblocks - 1))
                            # U node-major = psum * dis (per-partition scale)
                            um = umpool.tile([SW, D], bf16, tag="um")
                            nc.scalar.activation(
                                um[:w, :], ps[:w, :],
                                mybir.ActivationFunctionType.Copy,
                                scale=dis_pt[:w, tt:tt + 1])
                            pend.append((tt, w, um))
                            if len(pend) == GW:
                                flush_group(pend, gdone)
                                pend = []
                                gdone += 1
                    if pend:
                        flush_group(pend, gdone)
                        pend = []
                        gdone += 1

                    # ---- global BN stats (local reduction — no collective)
                    st = wpool.tile([D, 2], f32, tag="st")
                    nc.vector.reduce_sum(st[:, 0:1], sums[:],
                                         axis=mybir.AxisListType.X)
                    nc.vector.reduce_sum(st[:, 1:2], sums2[:],
                                         axis=mybir.AxisListType.X)
                    mu = wpool.tile([D, 1], f32, tag="mu")
                    nc.scalar.activation(mu[:], st[:, 0:1],
                                         mybir.ActivationFunctionType.Copy,
                                         scale=1.0 / n_nodes)
                    va = wpool.tile([D, 1], f32, tag="va")
                    nc.scalar.activation(va[:], st[:, 1:2],
                                         mybir.ActivationFunctionType.Copy,
                                         scale=1.0 / n_nodes)
                    mu2 = wpool.tile([D, 1], f32, tag="mu2")
                    nc.vector.tensor_tensor(out=mu2[:], in0=mu[:], in1=mu[:],
                                            op=mybir.AluOpType.mult)
                    nc.vector.tensor_tensor(out=va[:], in0=va[:], in1=mu2[:],
                                            op=mybir.AluOpType.subtract)
                    nc.scalar.activation(va[:], va[:],
                                         mybir.ActivationFunctionType.Sqrt,
                                         bias=eps_sb[:])
                    nc.vector.reciprocal(va[:], va[:])
                    saff = wpool.tile([D, 1], f32, tag="saff")
                    nc.vector.tensor_tensor(out=saff[:], in0=gamma_sb[:],
                                            in1=va[:], op=mybir.AluOpType.mult)
                    tsh_ = wpool.tile([D, 1], f32, tag="tsh")
                    nc.vector.tensor_tensor(out=tsh_[:], in0=mu[:], in1=saff[:],
                                            op=mybir.AluOpType.mult)
                    nc.vector.tensor_tensor(out=tsh_[:], in0=beta_sb[:],
                                            in1=tsh_[:],
                                            op=mybir.AluOpType.subtract)

                    # ---- activation phase per group
                    first_seen = set()
                    for gr in range(ngrp):
                        a = gr * GW * SW
                        b = min((gr + 1) * GW * SW, n_nodes)
                        gw = b - a
                        zt2 = wpool.tile([D, RW], bf16, tag="zt2")
                        nc.sync.dma_start(out=zt2[:, :gw],
                                          in_=z_dram[:, a:a + gw])
                        at = wpool.tile([D, RW], bf16, tag="at")
                        nc.scalar.activation(at[:, :gw], zt2[:, :gw],
                                             mybir.ActivationFunctionType.Relu,
                                             bias=tsh_[:], scale=saff[:])
                        if not last:
                            for tt in range(gr * GW, min((gr + 1) * GW, ntile)):
                                w = tsz[tt]
                                o = tt * SW - a
                                ptr = ps_tru.tile([SW, D], bf16,
                                                  tag="trp", space="PSUM")
                                nc.tensor.transpose(ptr[:w, :],
                                                    at[:, o:o + w],
                                                    identP[:D, :D])
                                wr = wpool.tile([SW, D], bf16, tag="wr")
                                nc.scalar.activation(
                                    wr[:w, :], ptr[:w, :],
                                    mybir.ActivationFunctionType.Copy,
                                    scale=dis_pt[:w, tt:tt + 1])
                                nc.sync.dma_start(
                                    out=table[tt * SW:tt * SW + w, 0:D],
                                    in_=wr[:w, :])
                        elif rep == reps - 1:
                            # pool graph segments on the fly
                            for (gr_, s0, s1, g) in cfg["pool_segs"]:
                                if gr_ != gr:
                                    continue
                                tmp = wpool.tile([D, 1], f32, tag="ptmp")
                                nc.vector.reduce_max(
                                    tmp[:], at[:, s0:s1],
                                    axis=mybir.AxisListType.X)
                                if g not in first_seen:
                                    first_seen.add(g)
                                    nc.vector.tensor_copy(emb[:, g:g + 1],
                                                          tmp[:])
                                else:
                                    nc.vector.tensor_tensor(
                                        out=emb[:, g:g + 1],
                                        in0=emb[:, g:g + 1], in1=tmp[:],
                                        op=mybir.AluOpType.max)

            # ---- head (all graphs, every core)
            emb_bf = wpool.tile([D, n_graphs], bf16, tag="embbf")
            nc.vector.tensor_copy(emb_bf[:], emb[:])
            ph_full = ps_z.tile([D, RW], f32, tag="zt", space="PSUM")
            ph = ph_full[:, :n_graphs]
            nc.tensor.matmul(ph, lhsT=l1w_sb[:], rhs=emb_bf[:],
                             start=True, stop=True)
            h1 = wpool.tile([D, n_graphs], bf16, tag="h1")
            nc.scalar.activation(h1[:], ph,
                                 mybir.ActivationFunctionType.Relu,
                                 bias=l1b_sb[:])
            po_full = ps_z.tile([D, RW], f32, tag="zt", space="PSUM")
            po = po_full[:ncls, :n_graphs]
            nc.tensor.matmul(po, lhsT=l2w_sb[:], rhs=h1[:],
                             start=True, stop=True)
            osb = wpool.tile([ncls, n_graphs], f32, tag="osb")
            nc.scalar.activation(osb[:], po,
                                 mybir.ActivationFunctionType.Identity,
                                 bias=l2b_sb[:])
            nc.sync.dma_start(out=out[:, :].rearrange("g c -> c g"), in_=osb[:])

    nc.compile()
    return nc


# ---------------------------------------------------------------- entry point

_CACHE = {}


def _get_built(cfg_key, cfg, reps, n_devices=N_CORES):
    key = (cfg_key, reps, n_devices)
    if key not in _CACHE:
        _CACHE[key] = _build(cfg, reps=reps, n_devices=n_devices)
    return _CACHE[key]


def _make_in_maps(cfg, data, x, inputs, n_devices=N_CORES):
    ncls = cfg["n_classes"]
    W_bf = [np.asarray(inputs[k], np.float32).astype(ml_dtypes.bfloat16)
            for k in ("W1", "W2", "W3")]
    m = {
        "x": x.astype(np.float32),
        "S": data["S"],
        "idx0": data["idx0"],
        "idx1": data["idx1"],
        "deg_pt": data["deg_pt"],
        "W1": W_bf[0], "W2": W_bf[1], "W3": W_bf[2],
        "gamma": np.asarray(inputs["gamma"], np.float32).reshape(D, 1),
        "beta": np.asarray(inputs["beta"], np.float32).reshape(D, 1),
        "lin1w": np.asarray(inputs["lin1_w"],
                            np.float32).astype(ml_dtypes.bfloat16),
        "lin1b": np.asarray(inputs["lin1_b"], np.float32).reshape(D, 1),
        "lin2w": np.asarray(inputs["lin2_w"],
                            np.float32).astype(ml_dtypes.bfloat16),
        "lin2b": np.asarray(inputs["lin2_b"], np.float32).reshape(ncls, 1),
    }
    return [m for _ in range(n_devices)]


def kernel(x, edge_index, batch, W1, b1, W2, b2, W3, b3, gamma, beta,
           lin1_w, lin1_b, lin2_w, lin2_b, _reps=1, _ndev=1):
    x = np.asarray(x, np.float32)
    edge_index = np.asarray(edge_index)
    batch = np.asarray(batch)
    n_nodes, d = x.shape
    ncls = np.asarray(lin2_w).shape[1]
    assert d == D

    cfg, data = _prep(x, edge_index, batch, ncls)

    # NOTE: b1/b2/b3 cancel inside BatchNorm (mean subtraction) - unused.
    in_maps = _make_in_maps(cfg, data, x, {
        "W1": W1, "W2": W2, "W3": W3, "gamma": gamma, "beta": beta,
        "lin1_w": lin1_w, "lin1_b": lin1_b, "lin2_w": lin2_w,
        "lin2_b": lin2_b}, n_devices=_ndev)

    cfg_key = (n_nodes, edge_index.shape[1], ncls, cfg["NBLK"])
    nc = _get_built(cfg_key, cfg, _reps, _ndev)
    res = run_bass_kernel_spmd(nc, in_maps, core_ids=list(range(_ndev)))
    return np.asarray(res.results[0]["out"]).astype(np.float32)


# revision 15
# speedup vs baseline: 1.1084x; 1.1084x over previous
"""Trainium2 Bass kernel for nn_GCN1PoolNorm: 3-layer GCN + shared BatchNorm +
global max pool + MLP head.

Self-contained: takes FULL inputs, returns FULL output [N_GRAPHS, N_CLASSES].

v3 design — FULLY REPLICATED, ZERO COLLECTIVES:
On this runner every collective costs ~27-38 ms (software-emulated NRT), so
any sharded design is collective-bound. Instead every core runs the whole
graph; the host takes core 0's output. No cross-core traffic at all.

Per layer (full graph, N=50000 nodes, E=850k edges incl. self loops):
- Node features h_tilde = act * dis live in a local DRAM table [N, 128] bf16
  (cols 0:64 = payload, 64:128 = zero pad -> 256B rows for SWDGE dma_gather).
- Host sorts edges by (dst window of 128, src half, src), pads each
  (window, half) group to 128-edge blocks. Per chunk of WC windows:
  2 dma_gathers (int16 idx limit -> src < 32768 and rest) fetch h_tilde[src]
  rows edge-major; S one-hot blocks [128e, 128d] fp8 stream from DRAM.
- Aggregation per window: chained PE matmuls psum[128d, 64f] +=
  S_blk.T @ msgs_blk[:, 0:64]  (node-major).
- Downstream per 4-window group: U = ACT(psum) * dis (per-partition scale),
  PE transpose to feat-major, Z = W.T @ U, BN stats via ACT accum_out
  (locally -> global stats without any AllReduce), z spilled to DRAM.
- Act phase per group: z -> BN affine + relu -> (layers 1,2) transpose back,
  * dis, write table rows; (layer 3) pool graph segments on the fly.
- Head computed for all 64 graphs on every core.
"""
import numpy as np
import ml_dtypes

from concourse import bacc, mybir, tile
from concourse.bass_utils import run_bass_kernel_spmd
from concourse.masks import make_identity

f32 = mybir.dt.float32
bf16 = mybir.dt.bfloat16
fp8 = mybir.dt.float8e4
i16 = mybir.dt.int16

N_CORES = 8
P = 128          # partition / block / dst-window quantum
D = 64           # feature dim
HALF = 32768     # int16 gather index limit
TROW = 128       # table row width (64 feats + 64 zero pad) -> 256B rows
BN_EPS = 1e-5
WC = 8           # dst windows per stream chunk
GW = 4           # windows per downstream group (512 nodes)


# ---------------------------------------------------------------- host prep

def _prep(x, edge_index, batch, n_classes):
    n_nodes = x.shape[0]
    n_graphs = int(batch.max()) + 1
    ntile = (n_nodes + P - 1) // P              # dst windows
    tsz = [min(P, n_nodes - t * P) for t in range(ntile)]

    src = edge_index[0].astype(np.int64)
    dst = edge_index[1].astype(np.int64)
    deg = np.bincount(dst, minlength=n_nodes).astype(np.int64)

    # edge stream (incl. self loops), sorted by (window, half, src)
    s = np.concatenate([src, np.arange(n_nodes)])
    dl = np.concatenate([dst, np.arange(n_nodes)])
    t = dl // P
    h = (s >= HALF).astype(np.int64)
    col = dl % P
    order = np.lexsort((s, h, t))
    s, t, h, col = s[order], t[order], h[order], col[order]
    cnts = np.zeros((ntile, 2), np.int64)
    np.add.at(cnts, (t, h), 1)

    nb = (cnts + P - 1) // P                    # [ntile, 2] blocks per group
    NB0 = int(nb[:, 0].sum())
    NB1 = int(nb[:, 1].sum())
    NBLK = NB0 + NB1
    cum0 = np.concatenate([[0], np.cumsum(nb[:, 0])]).astype(int)
    cum1 = np.concatenate([[0], np.cumsum(nb[:, 1])]).astype(int)
    sb = np.concatenate([[0], np.cumsum(nb.sum(axis=1))]).astype(int)

    idx0 = np.zeros(NB0 * P, np.int16)
    idx1 = np.zeros(NB1 * P, np.int16)
    S = np.zeros((P, NBLK, P), ml_dtypes.float8_e4m3)
    off = 0
    for tt in range(ntile):
        for hh in (0, 1):
            n = int(cnts[tt, hh])
            e = slice(off, off + n)
            off += n
            q = np.arange(n)
            if hh == 0:
                idx0[cum0[tt] * P:cum0[tt] * P + n] = s[e]
                blk0 = sb[tt]
            else:
                idx1[cum1[tt] * P:cum1[tt] * P + n] = s[e] - HALF
                blk0 = sb[tt] + nb[tt, 0]
            S[q % P, blk0 + q // P, col[e]] = 1.0
    # wrap indices: idx g -> [g % 16, g // 16], replicated on 128 partitions
    idx0w = np.tile(idx0.reshape(-1, 16).T, (8, 1))
    idx1w = np.tile(idx1.reshape(-1, 16).T, (8, 1))
    # pack per chunk: [idx0_chunk | idx1_chunk] contiguous -> 1 DMA per chunk
    chunks = [(t0, min(t0 + WC, ntile)) for t0 in range(0, ntile, WC)]
    segs = []
    for (t0, t1) in chunks:
        segs.append(idx0w[:, cum0[t0] * 8:cum0[t1] * 8])
        segs.append(idx1w[:, cum1[t0] * 8:cum1[t1] * 8])
    idxc = np.concatenate(segs, axis=1).copy()

    # deg layouts (fp32)
    deg_pt = np.zeros((P, ntile), np.float32)
    degf = deg.astype(np.float32)
    for tt in range(ntile):
        deg_pt[:tsz[tt], tt] = degf[tt * P:tt * P + tsz[tt]]

    # pooling segments grouped by 512-node groups (for on-the-fly pooling)
    gb = np.searchsorted(batch, np.arange(n_graphs + 1))
    ngrp = (ntile + GW - 1) // GW
    pool_segs = []                              # (grp, s0, s1, g) rel to group
    for gr in range(ngrp):
        a, b = gr * GW * P, min((gr + 1) * GW * P, n_nodes)
        for g in range(n_graphs):
            s0, e0 = max(a, int(gb[g])), min(b, int(gb[g + 1]))
            if s0 < e0:
                pool_segs.append((gr, s0 - a, e0 - a, g))

    cfg = dict(n_nodes=n_nodes, ntile=ntile, tsz=tsz,
               nb=nb.tolist(), NB0=NB0, NB1=NB1, NBLK=NBLK,
               cum0=cum0.tolist(), cum1=cum1.tolist(), sb=sb.tolist(),
               pool_segs=pool_segs, ngrp=ngrp,
               n_classes=n_classes, n_graphs=n_graphs)
    data = dict(idxc=idxc, S=S, deg_pt=deg_pt)
    return cfg, data


# ---------------------------------------------------------------- device build

def _build(cfg, reps=1, n_devices=N_CORES):
    ntile, tsz = cfg["ntile"], cfg["tsz"]
    nb, NB0, NB1, NBLK = cfg["nb"], cfg["NB0"], cfg["NB1"], cfg["NBLK"]
    cum0, cum1, sb = cfg["cum0"], cfg["cum1"], cfg["sb"]
    ncls = cfg["n_classes"]
    n_nodes, n_graphs = cfg["n_nodes"], cfg["n_graphs"]
    ngrp = cfg["ngrp"]
    ntp = ntile * P
    RW = 512

    chunks = [(t0, min(t0 + WC, ntile)) for t0 in range(0, ntile, WC)]
    CB0 = max(cum0[t1] - cum0[t0] for t0, t1 in chunks)
    CB1 = max(cum1[t1] - cum1[t0] for t0, t1 in chunks)
    CBS = max(sb[t1] - sb[t0] for t0, t1 in chunks)

    nc = bacc.Bacc(trn_type="TRN2", target_bir_lowering=False, debug=False,
                   num_devices=n_devices)

    x_in = nc.dram_tensor("x", [n_nodes, D], f32, kind="ExternalInput").ap()
    S_in = nc.dram_tensor("S", [P, NBLK, P], fp8, kind="ExternalInput").ap()
    idxc_in = nc.dram_tensor("idxc", [P, (NB0 + NB1) * 8], i16,
                             kind="ExternalInput").ap()
    deg_pt = nc.dram_tensor("deg_pt", [P, ntile], f32, kind="ExternalInput").ap()
    Ws = [nc.dram_tensor(f"W{i}", [D, D], bf16, kind="ExternalInput").ap()
          for i in (1, 2, 3)]
    gamma = nc.dram_tensor("gamma", [D, 1], f32, kind="ExternalInput").ap()
    beta = nc.dram_tensor("beta", [D, 1], f32, kind="ExternalInput").ap()
    lin1w = nc.dram_tensor("lin1w", [D, D], bf16, kind="ExternalInput").ap()
    lin1b = nc.dram_tensor("lin1b", [D, 1], f32, kind="ExternalInput").ap()
    lin2w = nc.dram_tensor("lin2w", [D, ncls], bf16, kind="ExternalInput").ap()
    lin2b = nc.dram_tensor("lin2b", [ncls, 1], f32, kind="ExternalInput").ap()
    out = nc.dram_tensor("out", [n_graphs, ncls], f32,
                         kind="ExternalOutput").ap()

    table = nc.dram_tensor("table", [ntp, TROW], bf16).ap()
    z_dram = nc.dram_tensor("z_dram", [D, ntp], bf16).ap()

    with tile.TileContext(nc) as tc:
        with (
            tc.tile_pool(name="const", bufs=1) as cpool,
            tc.tile_pool(name="sbuf_s", bufs=2) as spool,
            tc.tile_pool(name="msgs", bufs=2) as mpool,
            tc.tile_pool(name="idxp", bufs=2) as ipool,
            tc.tile_pool(name="work", bufs=3) as wpool,
            tc.tile_pool(name="ump", bufs=6) as umpool,
            tc.tile_pool(name="psacc", bufs=2, space="PSUM") as ps_acc,
            tc.tile_pool(name="pstru", bufs=2, space="PSUM") as ps_tru,
            tc.tile_pool(name="psz", bufs=2, space="PSUM") as ps_z,
        ):
            # ---- residents
            dis_pt = cpool.tile([P, ntile], f32)
            sums = cpool.tile([D, ngrp], f32)
            sums2 = cpool.tile([D, ngrp], f32)
            W_sb = [cpool.tile([D, D], bf16, tag=f"W{i}", name=f"W{i}_sb")
                    for i in range(3)]
            for i in range(3):
                nc.sync.dma_start(out=W_sb[i][:], in_=Ws[i][:])
            gamma_sb = cpool.tile([D, 1], f32, tag="gamma")
            beta_sb = cpool.tile([D, 1], f32, tag="beta")
            nc.sync.dma_start(out=gamma_sb[:], in_=gamma[:])
            nc.sync.dma_start(out=beta_sb[:], in_=beta[:])
            l1w_sb = cpool.tile([D, D], bf16, tag="l1w")
            l1b_sb = cpool.tile([D, 1], f32, tag="l1b")
            l2w_sb = cpool.tile([D, ncls], bf16, tag="l2w")
            l2b_sb = cpool.tile([ncls, 1], f32, tag="l2b")
            nc.sync.dma_start(out=l1w_sb[:], in_=lin1w[:])
            nc.sync.dma_start(out=l1b_sb[:], in_=lin1b[:])
            nc.sync.dma_start(out=l2w_sb[:], in_=lin2w[:])
            nc.sync.dma_start(out=l2b_sb[:], in_=lin2b[:])
            identP = cpool.tile([P, P], bf16, tag="identP")
            make_identity(nc, identP[:])
            emb = cpool.tile([D, n_graphs], f32, tag="emb")
            eps_sb = cpool.tile([D, 1], f32, tag="eps")
            nc.gpsimd.memset(eps_sb[:], BN_EPS)

            # one-time zero of table right half + pad rows
            zpad = cpool.tile([P, TROW], bf16, tag="zpad")
            nc.gpsimd.memset(zpad[:], 0.0)
            for tt in range(ntile):
                w = tsz[tt]
                nc.sync.dma_start(out=table[tt * P:tt * P + w, D:TROW],
                                  in_=zpad[:w, :D])
                if w < P:
                    nc.sync.dma_start(out=table[tt * P + w:(tt + 1) * P, :],
                                      in_=zpad[:P - w, :])

            # ---- dis (node-major per-partition layout only)
            dptf = wpool.tile([P, ntile], f32, tag="dptf")
            nc.sync.dma_start(out=dptf[:], in_=deg_pt[:])
            nc.scalar.activation(dis_pt[:], dptf[:],
                                 mybir.ActivationFunctionType.Sqrt, bias=1.0)
            nc.vector.reciprocal(dis_pt[:], dis_pt[:])

            # ---- table0 = bf16(x * dis)
            for tt in range(ntile):
                w = tsz[tt]
                xt = wpool.tile([P, D], f32, tag="xt")
                nc.sync.dma_start(out=xt[:w, :], in_=x_in[tt * P:tt * P + w, :])
                xb = wpool.tile([P, D], bf16, tag="xb")
                nc.scalar.activation(xb[:w, :], xt[:w, :],
                                     mybir.ActivationFunctionType.Copy,
                                     scale=dis_pt[:w, tt:tt + 1])
                nc.sync.dma_start(out=table[tt * P:tt * P + w, 0:D],
                                  in_=xb[:w, :])

            # ---- layers
            for rep in range(reps):
                for li in range(3):
                    last = (li == 2)
                    Wl = W_sb[li]
                    # aggregation + z, chunked
                    pend = []          # per-window node-major U psum tiles
                    gdone = 0

                    def flush_group(pend_tiles, gidx):
                        # pend_tiles: list of (t, w, um_tile)
                        ptru = ps_tru.tile([D, RW], bf16, tag="tru",
                                           space="PSUM")
                        gw = 0
                        for (tt_, w_, um_) in pend_tiles:
                            nc.tensor.transpose(ptru[:, gw:gw + w_],
                                                um_[:w_, :], identP[:w_, :w_])
                            gw += w_
                        ut = wpool.tile([D, RW], bf16, tag="ut")
                        nc.vector.tensor_copy(ut[:, :gw], ptru[:, :gw])
                        psz = ps_z.tile([D, RW], f32, tag="zt", space="PSUM")
                        nc.tensor.matmul(psz[:, :gw], lhsT=Wl[:],
                                         rhs=ut[:, :gw],
                                         start=True, stop=True)
                        zt = wpool.tile([D, RW], bf16, tag="ztile")
                        nc.scalar.activation(
                            zt[:, :gw], psz[:, :gw],
                            mybir.ActivationFunctionType.Copy,
                            accum_out=sums[:, gidx:gidx + 1])
                        sq = wpool.tile([D, RW], f32, tag="sq")
                        nc.scalar.activation(
                            sq[:, :gw], psz[:, :gw],
                            mybir.ActivationFunctionType.Square,
                            accum_out=sums2[:, gidx:gidx + 1])
                        g0 = pend_tiles[0][0] * P
                        nc.sync.dma_start(out=z_dram[:, g0:g0 + gw],
                                          in_=zt[:, :gw])

                    for (t0, t1) in chunks:
                        nb0c = cum0[t1] - cum0[t0]
                        nb1c = cum1[t1] - cum1[t0]
                        nbsc = sb[t1] - sb[t0]
                        S_t = spool.tile([P, CBS, P], fp8, tag="S")
                        nc.sync.dma_start(out=S_t[:, :nbsc, :],
                                          in_=S_in[:, sb[t0]:sb[t1], :])
                        ixoff = (cum0[t0] + cum1[t0]) * 8
                        ixw = (nb0c + nb1c) * 8
                        ixc = ipool.tile([P, (CB0 + CB1) * 8], i16, tag="ixc")
                        nc.sync.dma_start(
                            out=ixc[:, :ixw],
                            in_=idxc_in[:, ixoff:ixoff + ixw])
                        m0 = mpool.tile([P, CB0, TROW], bf16, tag="m0")
                        if nb0c:
                            nc.gpsimd.dma_gather(
                                m0[:, :nb0c, :], table[:HALF, :],
                                ixc[:, :nb0c * 8],
                                nb0c * P, nb0c * P, TROW,
                                single_packet=False)
                        m1 = mpool.tile([P, CB1, TROW], bf16, tag="m1")
                        if nb1c:
                            nc.gpsimd.dma_gather(
                                m1[:, :nb1c, :], table[HALF:ntp, :],
                                ixc[:, nb0c * 8:ixw],
                                nb1c * P, nb1c * P, TROW,
                                single_packet=False)
                        for tt in range(t0, t1):
                            w = tsz[tt]
                            nblocks = nb[tt][0] + nb[tt][1]
                            ps = ps_acc.tile([P, D], f32, tag="acc",
                                             space="PSUM")
                            for i in range(nblocks):
                                if i < nb[tt][0]:
                                    rhs = m0[:, cum0[tt] - cum0[t0] + i, 0:D]
                                else:
                                    rhs = m1[:, cum1[tt] - cum1[t0]
                                             + (i - nb[tt][0]), 0:D]
                                lhs = S_t[:, sb[tt] - sb[t0] + i, :]
                                nc.tensor.matmul(
                                    ps[:], lhsT=lhs, rhs=rhs,
                                    start=(i == 0), stop=(i == nblocks - 1))
                            # U node-major = psum * dis (per-partition scale)
                            um = umpool.tile([P, D], bf16, tag="um")
                            nc.scalar.activation(
                                um[:w, :], ps[:w, :],
                                mybir.ActivationFunctionType.Copy,
                                scale=dis_pt[:w, tt:tt + 1])
                            pend.append((tt, w, um))
                            if len(pend) == GW:
                                flush_group(pend, gdone)
                                pend = []
                                gdone += 1
                    if pend:
                        flush_group(pend, gdone)
                        pend = []
                        gdone += 1

                    # ---- global BN stats (local reduction — no collective)
                    st = wpool.tile([D, 2], f32, tag="st")
                    nc.vector.reduce_sum(st[:, 0:1], sums[:],
                                         axis=mybir.AxisListType.X)
                    nc.vector.reduce_sum(st[:, 1:2], sums2[:],
                                         axis=mybir.AxisListType.X)
                    mu = wpool.tile([D, 1], f32, tag="mu")
                    nc.scalar.activation(mu[:], st[:, 0:1],
                                         mybir.ActivationFunctionType.Copy,
                                         scale=1.0 / n_nodes)
                    va = wpool.tile([D, 1], f32, tag="va")
                    nc.scalar.activation(va[:], st[:, 1:2],
                                         mybir.ActivationFunctionType.Copy,
                                         scale=1.0 / n_nodes)
                    mu2 = wpool.tile([D, 1], f32, tag="mu2")
                    nc.vector.tensor_tensor(out=mu2[:], in0=mu[:], in1=mu[:],
                                            op=mybir.AluOpType.mult)
                    nc.vector.tensor_tensor(out=va[:], in0=va[:], in1=mu2[:],
                                            op=mybir.AluOpType.subtract)
                    nc.scalar.activation(va[:], va[:],
                                         mybir.ActivationFunctionType.Sqrt,
                                         bias=eps_sb[:])
                    nc.vector.reciprocal(va[:], va[:])
                    saff = wpool.tile([D, 1], f32, tag="saff")
                    nc.vector.tensor_tensor(out=saff[:], in0=gamma_sb[:],
                                            in1=va[:], op=mybir.AluOpType.mult)
                    tsh_ = wpool.tile([D, 1], f32, tag="tsh")
                    nc.vector.tensor_tensor(out=tsh_[:], in0=mu[:], in1=saff[:],
                                            op=mybir.AluOpType.mult)
                    nc.vector.tensor_tensor(out=tsh_[:], in0=beta_sb[:],
                                            in1=tsh_[:],
                                            op=mybir.AluOpType.subtract)

                    # ---- activation phase per group
                    first_seen = set()
                    for gr in range(ngrp):
                        a = gr * GW * P
                        b = min((gr + 1) * GW * P, n_nodes)
                        gw = b - a
                        zt2 = wpool.tile([D, RW], bf16, tag="zt2")
                        nc.sync.dma_start(out=zt2[:, :gw],
                                          in_=z_dram[:, a:a + gw])
                        at = wpool.tile([D, RW], bf16, tag="at")
                        nc.scalar.activation(at[:, :gw], zt2[:, :gw],
                                             mybir.ActivationFunctionType.Relu,
                                             bias=tsh_[:], scale=saff[:])
                        if not last:
                            for tt in range(gr * GW, min((gr + 1) * GW, ntile)):
                                w = tsz[tt]
                                o = tt * P - a
                                ptr = ps_tru.tile([P, D], bf16,
                                                  tag="trp", space="PSUM")
                                nc.tensor.transpose(ptr[:w, :],
                                                    at[:, o:o + w],
                                                    identP[:D, :D])
                                wr = wpool.tile([P, D], bf16, tag="wr")
                                nc.scalar.activation(
                                    wr[:w, :], ptr[:w, :],
                                    mybir.ActivationFunctionType.Copy,
                                    scale=dis_pt[:w, tt:tt + 1])
                                nc.sync.dma_start(
                                    out=table[tt * P:tt * P + w, 0:D],
                                    in_=wr[:w, :])
                        elif rep == reps - 1:
                            # pool graph segments on the fly
                            for (gr_, s0, s1, g) in cfg["pool_segs"]:
                                if gr_ != gr:
                                    continue
                                tmp = wpool.tile([D, 1], f32, tag="ptmp")
                                nc.vector.reduce_max(
                                    tmp[:], at[:, s0:s1],
                                    axis=mybir.AxisListType.X)
                                if g not in first_seen:
                                    first_seen.add(g)
                                    nc.vector.tensor_copy(emb[:, g:g + 1],
                                                          tmp[:])
                                else:
                                    nc.vector.tensor_tensor(
                                        out=emb[:, g:g + 1],
                                        in0=emb[:, g:g + 1], in1=tmp[:],
                                        op=mybir.AluOpType.max)

            # ---- head (all graphs, every core)
            emb_bf = wpool.tile([D, n_graphs], bf16, tag="embbf")
            nc.vector.tensor_copy(emb_bf[:], emb[:])
            ph_full = ps_z.tile([D, RW], f32, tag="zt", space="PSUM")
            ph = ph_full[:, :n_graphs]
            nc.tensor.matmul(ph, lhsT=l1w_sb[:], rhs=emb_bf[:],
                             start=True, stop=True)
            h1 = wpool.tile([D, n_graphs], bf16, tag="h1")
            nc.scalar.activation(h1[:], ph,
                                 mybir.ActivationFunctionType.Relu,
                                 bias=l1b_sb[:])
            po_full = ps_z.tile([D, RW], f32, tag="zt", space="PSUM")
            po = po_full[:ncls, :n_graphs]
            nc.tensor.matmul(po, lhsT=l2w_sb[:], rhs=h1[:],
                             start=True, stop=True)
            osb = wpool.tile([ncls, n_graphs], f32, tag="osb")
            nc.scalar.activation(osb[:], po,
                                 mybir.ActivationFunctionType.Identity,
                                 bias=l2b_sb[:])
            nc.sync.dma_start(out=out[:, :].rearrange("g c -> c g"), in_=osb[:])

    nc.compile()
    return nc


# ---------------------------------------------------------------- entry point

_CACHE = {}


def _get_built(cfg_key, cfg, reps, n_devices=N_CORES):
    key = (cfg_key, reps, n_devices)
    if key not in _CACHE:
        _CACHE[key] = _build(cfg, reps=reps, n_devices=n_devices)
    return _CACHE[key]


def _make_in_maps(cfg, data, x, inputs, n_devices=N_CORES):
    ncls = cfg["n_classes"]
    W_bf = [np.asarray(inputs[k], np.float32).astype(ml_dtypes.bfloat16)
            for k in ("W1", "W2", "W3")]
    m = {
        "x": x.astype(np.float32),
        "S": data["S"],
        "idxc": data["idxc"],
        "deg_pt": data["deg_pt"],
        "W1": W_bf[0], "W2": W_bf[1], "W3": W_bf[2],
        "gamma": np.asarray(inputs["gamma"], np.float32).reshape(D, 1),
        "beta": np.asarray(inputs["beta"], np.float32).reshape(D, 1),
        "lin1w": np.asarray(inputs["lin1_w"],
                            np.float32).astype(ml_dtypes.bfloat16),
        "lin1b": np.asarray(inputs["lin1_b"], np.float32).reshape(D, 1),
        "lin2w": np.asarray(inputs["lin2_w"],
                            np.float32).astype(ml_dtypes.bfloat16),
        "lin2b": np.asarray(inputs["lin2_b"], np.float32).reshape(ncls, 1),
    }
    return [m for _ in range(n_devices)]


def kernel(x, edge_index, batch, W1, b1, W2, b2, W3, b3, gamma, beta,
           lin1_w, lin1_b, lin2_w, lin2_b, _reps=1, _ndev=1):
    x = np.asarray(x, np.float32)
    edge_index = np.asarray(edge_index)
    batch = np.asarray(batch)
    n_nodes, d = x.shape
    ncls = np.asarray(lin2_w).shape[1]
    assert d == D

    cfg, data = _prep(x, edge_index, batch, ncls)

    # NOTE: b1/b2/b3 cancel inside BatchNorm (mean subtraction) - unused.
    in_maps = _make_in_maps(cfg, data, x, {
        "W1": W1, "W2": W2, "W3": W3, "gamma": gamma, "beta": beta,
        "lin1_w": lin1_w, "lin1_b": lin1_b, "lin2_w": lin2_w,
        "lin2_b": lin2_b}, n_devices=_ndev)

    cfg_key = (n_nodes, edge_index.shape[1], ncls, cfg["NBLK"])
    nc = _get_built(cfg_key, cfg, _reps, _ndev)
    res = run_bass_kernel_spmd(nc, in_maps, core_ids=list(range(_ndev)))
    return np.asarray(res.results[0]["out"]).astype(np.float32)


# revision 17
# speedup vs baseline: 1.9458x; 1.7555x over previous
"""Trainium2 Bass kernel for nn_GCN1PoolNorm: 3-layer GCN + shared BatchNorm +
global max pool + MLP head.

Self-contained: takes FULL inputs, returns FULL output [N_GRAPHS, N_CLASSES].

v3 design — FULLY REPLICATED, ZERO COLLECTIVES:
On this runner every collective costs ~27-38 ms (software-emulated NRT), so
any sharded design is collective-bound. Instead every core runs the whole
graph; the host takes core 0's output. No cross-core traffic at all.

Per layer (full graph, N=50000 nodes, E=850k edges incl. self loops):
- Node features h_tilde = act * dis live in a local DRAM table [N, 128] bf16
  (cols 0:64 = payload, 64:128 = zero pad -> 256B rows for SWDGE dma_gather).
- Host sorts edges by (dst window of 128, src half, src), pads each
  (window, half) group to 128-edge blocks. Per chunk of WC windows:
  2 dma_gathers (int16 idx limit -> src < 32768 and rest) fetch h_tilde[src]
  rows edge-major; S one-hot blocks [128e, 128d] fp8 stream from DRAM.
- Aggregation per window: chained PE matmuls psum[128d, 64f] +=
  S_blk.T @ msgs_blk[:, 0:64]  (node-major).
- Downstream per 4-window group: U = ACT(psum) * dis (per-partition scale),
  PE transpose to feat-major, Z = W.T @ U, BN stats via ACT accum_out
  (locally -> global stats without any AllReduce), z spilled to DRAM.
- Act phase per group: z -> BN affine + relu -> (layers 1,2) transpose back,
  * dis, write table rows; (layer 3) pool graph segments on the fly.
- Head computed for all 64 graphs on every core.
"""
import numpy as np
import ml_dtypes

from concourse import bacc, mybir, tile
from concourse.bass_utils import run_bass_kernel_spmd
from concourse.masks import make_identity

f32 = mybir.dt.float32
bf16 = mybir.dt.bfloat16
fp8 = mybir.dt.float8e4
i16 = mybir.dt.int16

N_CORES = 8
P = 128          # partition / block / dst-window quantum
D = 64           # feature dim
HALF = 32768     # int16 gather index limit
TROW = 128       # table row width (64 feats + 64 zero pad) -> 256B rows
BN_EPS = 1e-5
WC = 8           # dst windows per stream chunk
GW = 4           # windows per downstream group (512 nodes)


# ---------------------------------------------------------------- host prep

def _prep(x, edge_index, batch, n_classes):
    n_nodes = x.shape[0]
    n_graphs = int(batch.max()) + 1
    ntile = (n_nodes + P - 1) // P              # dst windows
    tsz = [min(P, n_nodes - t * P) for t in range(ntile)]

    src = edge_index[0].astype(np.int64)
    dst = edge_index[1].astype(np.int64)
    deg = np.bincount(dst, minlength=n_nodes).astype(np.int64)

    # edge stream (incl. self loops), sorted by (window, half, src)
    s = np.concatenate([src, np.arange(n_nodes)])
    dl = np.concatenate([dst, np.arange(n_nodes)])
    t = dl // P
    h = (s >= HALF).astype(np.int64)
    col = dl % P
    order = np.lexsort((s, h, t))
    s, t, h, col = s[order], t[order], h[order], col[order]
    cnts = np.zeros((ntile, 2), np.int64)
    np.add.at(cnts, (t, h), 1)

    nb = (cnts + P - 1) // P                    # [ntile, 2] blocks per group
    NB0 = int(nb[:, 0].sum())
    NB1 = int(nb[:, 1].sum())
    NBLK = NB0 + NB1
    cum0 = np.concatenate([[0], np.cumsum(nb[:, 0])]).astype(int)
    cum1 = np.concatenate([[0], np.cumsum(nb[:, 1])]).astype(int)
    sb = np.concatenate([[0], np.cumsum(nb.sum(axis=1))]).astype(int)

    idx0 = np.zeros(NB0 * P, np.int16)
    idx1 = np.zeros(NB1 * P, np.int16)
    S = np.zeros((P, NBLK, P), ml_dtypes.float8_e4m3)
    off = 0
    for tt in range(ntile):
        for hh in (0, 1):
            n = int(cnts[tt, hh])
            e = slice(off, off + n)
            off += n
            q = np.arange(n)
            if hh == 0:
                idx0[cum0[tt] * P:cum0[tt] * P + n] = s[e]
                blk0 = sb[tt]
            else:
                idx1[cum1[tt] * P:cum1[tt] * P + n] = s[e] - HALF
                blk0 = sb[tt] + nb[tt, 0]
            S[q % P, blk0 + q // P, col[e]] = 1.0
    # wrap indices: idx g -> [g % 16, g // 16], replicated on 128 partitions
    idx0w = np.tile(idx0.reshape(-1, 16).T, (8, 1)).copy()
    idx1w = np.tile(idx1.reshape(-1, 16).T, (8, 1)).copy()

    # deg layouts (fp32)
    deg_pt = np.zeros((P, ntile), np.float32)
    degf = deg.astype(np.float32)
    for tt in range(ntile):
        deg_pt[:tsz[tt], tt] = degf[tt * P:tt * P + tsz[tt]]

    # pooling segments grouped by 512-node groups (for on-the-fly pooling)
    gb = np.searchsorted(batch, np.arange(n_graphs + 1))
    ngrp = (ntile + GW - 1) // GW
    pool_segs = []                              # (grp, s0, s1, g) rel to group
    for gr in range(ngrp):
        a, b = gr * GW * P, min((gr + 1) * GW * P, n_nodes)
        for g in range(n_graphs):
            s0, e0 = max(a, int(gb[g])), min(b, int(gb[g + 1]))
            if s0 < e0:
                pool_segs.append((gr, s0 - a, e0 - a, g))

    cfg = dict(n_nodes=n_nodes, ntile=ntile, tsz=tsz,
               nb=nb.tolist(), NB0=NB0, NB1=NB1, NBLK=NBLK,
               cum0=cum0.tolist(), cum1=cum1.tolist(), sb=sb.tolist(),
               pool_segs=pool_segs, ngrp=ngrp,
               n_classes=n_classes, n_graphs=n_graphs)
    data = dict(idx0=idx0w, idx1=idx1w, S=S, deg_pt=deg_pt)
    return cfg, data


# ---------------------------------------------------------------- device build

def _build(cfg, reps=1, n_devices=N_CORES):
    ntile, tsz = cfg["ntile"], cfg["tsz"]
    nb, NB0, NB1, NBLK = cfg["nb"], cfg["NB0"], cfg["NB1"], cfg["NBLK"]
    cum0, cum1, sb = cfg["cum0"], cfg["cum1"], cfg["sb"]
    ncls = cfg["n_classes"]
    n_nodes, n_graphs = cfg["n_nodes"], cfg["n_graphs"]
    ngrp = cfg["ngrp"]
    ntp = ntile * P
    RW = 512

    chunks = [(t0, min(t0 + WC, ntile)) for t0 in range(0, ntile, WC)]
    CB0 = max(cum0[t1] - cum0[t0] for t0, t1 in chunks)
    CB1 = max(cum1[t1] - cum1[t0] for t0, t1 in chunks)
    CBS = max(sb[t1] - sb[t0] for t0, t1 in chunks)

    nc = bacc.Bacc(trn_type="TRN2", target_bir_lowering=False, debug=False,
                   num_devices=n_devices, num_swdge_queues=2,
                   dynamic_dma_scratch_size=32768)

    x_in = nc.dram_tensor("x", [n_nodes, D], f32, kind="ExternalInput").ap()
    S_in = nc.dram_tensor("S", [P, NBLK, P], fp8, kind="ExternalInput").ap()
    idx0_in = nc.dram_tensor("idx0", [P, NB0 * 8], i16, kind="ExternalInput").ap()
    idx1_in = nc.dram_tensor("idx1", [P, NB1 * 8], i16, kind="ExternalInput").ap()
    deg_pt = nc.dram_tensor("deg_pt", [P, ntile], f32, kind="ExternalInput").ap()
    Ws = [nc.dram_tensor(f"W{i}", [D, D], bf16, kind="ExternalInput").ap()
          for i in (1, 2, 3)]
    gamma = nc.dram_tensor("gamma", [D, 1], f32, kind="ExternalInput").ap()
    beta = nc.dram_tensor("beta", [D, 1], f32, kind="ExternalInput").ap()
    lin1w = nc.dram_tensor("lin1w", [D, D], bf16, kind="ExternalInput").ap()
    lin1b = nc.dram_tensor("lin1b", [D, 1], f32, kind="ExternalInput").ap()
    lin2w = nc.dram_tensor("lin2w", [D, ncls], bf16, kind="ExternalInput").ap()
    lin2b = nc.dram_tensor("lin2b", [ncls, 1], f32, kind="ExternalInput").ap()
    out = nc.dram_tensor("out", [n_graphs, ncls], f32,
                         kind="ExternalOutput").ap()

    table = nc.dram_tensor("table", [ntp, TROW], bf16).ap()
    z_dram = nc.dram_tensor("z_dram", [D, ntp], bf16).ap()

    with tile.TileContext(nc) as tc:
        with (
            tc.tile_pool(name="const", bufs=1) as cpool,
            tc.tile_pool(name="sbuf_s", bufs=2) as spool,
            tc.tile_pool(name="msgs", bufs=2) as mpool,
            tc.tile_pool(name="idxp", bufs=2) as ipool,
            tc.tile_pool(name="work", bufs=3) as wpool,
            tc.tile_pool(name="ump", bufs=6) as umpool,
            tc.tile_pool(name="psacc", bufs=2, space="PSUM") as ps_acc,
            tc.tile_pool(name="pstru", bufs=2, space="PSUM") as ps_tru,
            tc.tile_pool(name="psz", bufs=2, space="PSUM") as ps_z,
        ):
            # ---- residents
            dis_pt = cpool.tile([P, ntile], f32)
            sums = cpool.tile([D, ngrp], f32)
            sums2 = cpool.tile([D, ngrp], f32)
            W_sb = [cpool.tile([D, D], bf16, tag=f"W{i}", name=f"W{i}_sb")
                    for i in range(3)]
            for i in range(3):
                nc.sync.dma_start(out=W_sb[i][:], in_=Ws[i][:])
            gamma_sb = cpool.tile([D, 1], f32, tag="gamma")
            beta_sb = cpool.tile([D, 1], f32, tag="beta")
            nc.sync.dma_start(out=gamma_sb[:], in_=gamma[:])
            nc.sync.dma_start(out=beta_sb[:], in_=beta[:])
            l1w_sb = cpool.tile([D, D], bf16, tag="l1w")
            l1b_sb = cpool.tile([D, 1], f32, tag="l1b")
            l2w_sb = cpool.tile([D, ncls], bf16, tag="l2w")
            l2b_sb = cpool.tile([ncls, 1], f32, tag="l2b")
            nc.sync.dma_start(out=l1w_sb[:], in_=lin1w[:])
            nc.sync.dma_start(out=l1b_sb[:], in_=lin1b[:])
            nc.sync.dma_start(out=l2w_sb[:], in_=lin2w[:])
            nc.sync.dma_start(out=l2b_sb[:], in_=lin2b[:])
            identP = cpool.tile([P, P], bf16, tag="identP")
            make_identity(nc, identP[:])
            emb = cpool.tile([D, n_graphs], f32, tag="emb")
            eps_sb = cpool.tile([D, 1], f32, tag="eps")
            nc.gpsimd.memset(eps_sb[:], BN_EPS)

            # one-time zero of table right half + pad rows
            zpad = cpool.tile([P, TROW], bf16, tag="zpad")
            nc.gpsimd.memset(zpad[:], 0.0)
            for tt in range(ntile):
                w = tsz[tt]
                nc.sync.dma_start(out=table[tt * P:tt * P + w, D:TROW],
                                  in_=zpad[:w, :D])
                if w < P:
                    nc.sync.dma_start(out=table[tt * P + w:(tt + 1) * P, :],
                                      in_=zpad[:P - w, :])

            # ---- dis (node-major per-partition layout only)
            dptf = wpool.tile([P, ntile], f32, tag="dptf")
            nc.sync.dma_start(out=dptf[:], in_=deg_pt[:])
            nc.scalar.activation(dis_pt[:], dptf[:],
                                 mybir.ActivationFunctionType.Sqrt, bias=1.0)
            nc.vector.reciprocal(dis_pt[:], dis_pt[:])

            # ---- table0 = bf16(x * dis)
            for tt in range(ntile):
                w = tsz[tt]
                xt = wpool.tile([P, D], f32, tag="xt")
                nc.sync.dma_start(out=xt[:w, :], in_=x_in[tt * P:tt * P + w, :])
                xb = wpool.tile([P, D], bf16, tag="xb")
                nc.scalar.activation(xb[:w, :], xt[:w, :],
                                     mybir.ActivationFunctionType.Copy,
                                     scale=dis_pt[:w, tt:tt + 1])
                nc.sync.dma_start(out=table[tt * P:tt * P + w, 0:D],
                                  in_=xb[:w, :])

            # ---- layers
            for rep in range(reps):
                for li in range(3):
                    last = (li == 2)
                    Wl = W_sb[li]
                    # aggregation + z, chunked
                    pend = []          # per-window node-major U psum tiles
                    gdone = 0

                    def flush_group(pend_tiles, gidx):
                        # pend_tiles: list of (t, w, um_tile)
                        ptru = ps_tru.tile([D, RW], bf16, tag="tru",
                                           space="PSUM")
                        gw = 0
                        for (tt_, w_, um_) in pend_tiles:
                            nc.tensor.transpose(ptru[:, gw:gw + w_],
                                                um_[:w_, :], identP[:w_, :w_])
                            gw += w_
                        ut = wpool.tile([D, RW], bf16, tag="ut")
                        nc.vector.tensor_copy(ut[:, :gw], ptru[:, :gw])
                        psz = ps_z.tile([D, RW], f32, tag="zt", space="PSUM")
                        nc.tensor.matmul(psz[:, :gw], lhsT=Wl[:],
                                         rhs=ut[:, :gw],
                                         start=True, stop=True)
                        zt = wpool.tile([D, RW], bf16, tag="ztile")
                        nc.scalar.activation(
                            zt[:, :gw], psz[:, :gw],
                            mybir.ActivationFunctionType.Copy,
                            accum_out=sums[:, gidx:gidx + 1])
                        sq = wpool.tile([D, RW], f32, tag="sq")
                        nc.scalar.activation(
                            sq[:, :gw], psz[:, :gw],
                            mybir.ActivationFunctionType.Square,
                            accum_out=sums2[:, gidx:gidx + 1])
                        g0 = pend_tiles[0][0] * P
                        nc.sync.dma_start(out=z_dram[:, g0:g0 + gw],
                                          in_=zt[:, :gw])

                    for (t0, t1) in chunks:
                        nb0c = cum0[t1] - cum0[t0]
                        nb1c = cum1[t1] - cum1[t0]
                        nbsc = sb[t1] - sb[t0]
                        S_t = spool.tile([P, CBS, P], fp8, tag="S")
                        nc.sync.dma_start(out=S_t[:, :nbsc, :],
                                          in_=S_in[:, sb[t0]:sb[t1], :])
                        m0 = mpool.tile([P, CB0, TROW], bf16, tag="m0")
                        if nb0c:
                            ix0 = ipool.tile([P, CB0 * 8], i16, tag="ix0")
                            nc.sync.dma_start(
                                out=ix0[:, :nb0c * 8],
                                in_=idx0_in[:, cum0[t0] * 8:cum0[t1] * 8])
                            nc.gpsimd.dma_gather(
                                m0[:, :nb0c, :], table[:HALF, :],
                                ix0[:, :nb0c * 8],
                                nb0c * P, nb0c * P, TROW,
                                single_packet=False)
                        m1 = mpool.tile([P, CB1, TROW], bf16, tag="m1")
                        if nb1c:
                            ix1 = ipool.tile([P, CB1 * 8], i16, tag="ix1")
                            nc.sync.dma_start(
                                out=ix1[:, :nb1c * 8],
                                in_=idx1_in[:, cum1[t0] * 8:cum1[t1] * 8])
                            nc.gpsimd.dma_gather(
                                m1[:, :nb1c, :], table[HALF:ntp, :],
                                ix1[:, :nb1c * 8],
                                nb1c * P, nb1c * P, TROW,
                                single_packet=False, queue_num=1)
                        for tt in range(t0, t1):
                            w = tsz[tt]
                            nblocks = nb[tt][0] + nb[tt][1]
                            ps = ps_acc.tile([P, D], f32, tag="acc",
                                             space="PSUM")
                            for i in range(nblocks):
                                if i < nb[tt][0]:
                                    rhs = m0[:, cum0[tt] - cum0[t0] + i, 0:D]
                                else:
                                    rhs = m1[:, cum1[tt] - cum1[t0]
                                             + (i - nb[tt][0]), 0:D]
                                lhs = S_t[:, sb[tt] - sb[t0] + i, :]
                                nc.tensor.matmul(
                                    ps[:], lhsT=lhs, rhs=rhs,
                                    start=(i == 0), stop=(i == nblocks - 1))
                            # U node-major = psum * dis (per-partition scale)
                            um = umpool.tile([P, D], bf16, tag="um")
                            nc.scalar.activation(
                                um[:w, :], ps[:w, :],
                                mybir.ActivationFunctionType.Copy,
                                scale=dis_pt[:w, tt:tt + 1])
                            pend.append((tt, w, um))
                            if len(pend) == GW:
                                flush_group(pend, gdone)
                                pend = []
                                gdone += 1
                    if pend:
                        flush_group(pend, gdone)
                        pend = []
                        gdone += 1

                    # ---- global BN stats (local reduction — no collective)
                    st = wpool.tile([D, 2], f32, tag="st")
                    nc.vector.reduce_sum(st[:, 0:1], sums[:],
                                         axis=mybir.AxisListType.X)
                    nc.vector.reduce_sum(st[:, 1:2], sums2[:],
                                         axis=mybir.AxisListType.X)
                    mu = wpool.tile([D, 1], f32, tag="mu")
                    nc.scalar.activation(mu[:], st[:, 0:1],
                                         mybir.ActivationFunctionType.Copy,
                                         scale=1.0 / n_nodes)
                    va = wpool.tile([D, 1], f32, tag="va")
                    nc.scalar.activation(va[:], st[:, 1:2],
                                         mybir.ActivationFunctionType.Copy,
                                         scale=1.0 / n_nodes)
                    mu2 = wpool.tile([D, 1], f32, tag="mu2")
                    nc.vector.tensor_tensor(out=mu2[:], in0=mu[:], in1=mu[:],
                                            op=mybir.AluOpType.mult)
                    nc.vector.tensor_tensor(out=va[:], in0=va[:], in1=mu2[:],
                                            op=mybir.AluOpType.subtract)
                    nc.scalar.activation(va[:], va[:],
                                         mybir.ActivationFunctionType.Sqrt,
                                         bias=eps_sb[:])
                    nc.vector.reciprocal(va[:], va[:])
                    saff = wpool.tile([D, 1], f32, tag="saff")
                    nc.vector.tensor_tensor(out=saff[:], in0=gamma_sb[:],
                                            in1=va[:], op=mybir.AluOpType.mult)
                    tsh_ = wpool.tile([D, 1], f32, tag="tsh")
                    nc.vector.tensor_tensor(out=tsh_[:], in0=mu[:], in1=saff[:],
                                            op=mybir.AluOpType.mult)
                    nc.vector.tensor_tensor(out=tsh_[:], in0=beta_sb[:],
                                            in1=tsh_[:],
                                            op=mybir.AluOpType.subtract)

                    # ---- activation phase per group
                    first_seen = set()
                    for gr in range(ngrp):
                        a = gr * GW * P
                        b = min((gr + 1) * GW * P, n_nodes)
                        gw = b - a
                        zt2 = wpool.tile([D, RW], bf16, tag="zt2")
                        nc.sync.dma_start(out=zt2[:, :gw],
                                          in_=z_dram[:, a:a + gw])
                        at = wpool.tile([D, RW], bf16, tag="at")
                        nc.scalar.activation(at[:, :gw], zt2[:, :gw],
                                             mybir.ActivationFunctionType.Relu,
                                             bias=tsh_[:], scale=saff[:])
                        if not last:
                            for tt in range(gr * GW, min((gr + 1) * GW, ntile)):
                                w = tsz[tt]
                                o = tt * P - a
                                ptr = ps_tru.tile([P, D], bf16,
                                                  tag="trp", space="PSUM")
                                nc.tensor.transpose(ptr[:w, :],
                                                    at[:, o:o + w],
                                                    identP[:D, :D])
                                wr = wpool.tile([P, D], bf16, tag="wr")
                                nc.scalar.activation(
                                    wr[:w, :], ptr[:w, :],
                                    mybir.ActivationFunctionType.Copy,
                                    scale=dis_pt[:w, tt:tt + 1])
                                nc.sync.dma_start(
                                    out=table[tt * P:tt * P + w, 0:D],
                                    in_=wr[:w, :])
                        elif rep == reps - 1:
                            # pool graph segments on the fly
                            for (gr_, s0, s1, g) in cfg["pool_segs"]:
                                if gr_ != gr:
                                    continue
                                tmp = wpool.tile([D, 1], f32, tag="ptmp")
                                nc.vector.reduce_max(
                                    tmp[:], at[:, s0:s1],
                                    axis=mybir.AxisListType.X)
                                if g not in first_seen:
                                    first_seen.add(g)
                                    nc.vector.tensor_copy(emb[:, g:g + 1],
                                                          tmp[:])
                                else:
                                    nc.vector.tensor_tensor(
                                        out=emb[:, g:g + 1],
                                        in0=emb[:, g:g + 1], in1=tmp[:],
                                        op=mybir.AluOpType.max)

            # ---- head (all graphs, every core)
            emb_bf = wpool.tile([D, n_graphs], bf16, tag="embbf")
            nc.vector.tensor_copy(emb_bf[:], emb[:])
            ph_full = ps_z.tile([D, RW], f32, tag="zt", space="PSUM")
            ph = ph_full[:, :n_graphs]
            nc.tensor.matmul(ph, lhsT=l1w_sb[:], rhs=emb_bf[:],
                             start=True, stop=True)
            h1 = wpool.tile([D, n_graphs], bf16, tag="h1")
            nc.scalar.activation(h1[:], ph,
                                 mybir.ActivationFunctionType.Relu,
                                 bias=l1b_sb[:])
            po_full = ps_z.tile([D, RW], f32, tag="zt", space="PSUM")
            po = po_full[:ncls, :n_graphs]
            nc.tensor.matmul(po, lhsT=l2w_sb[:], rhs=h1[:],
                             start=True, stop=True)
            osb = wpool.tile([ncls, n_graphs], f32, tag="osb")
            nc.scalar.activation(osb[:], po,
                                 mybir.ActivationFunctionType.Identity,
                                 bias=l2b_sb[:])
            nc.sync.dma_start(out=out[:, :].rearrange("g c -> c g"), in_=osb[:])

    nc.compile()
    return nc


# ---------------------------------------------------------------- entry point

_CACHE = {}


def _get_built(cfg_key, cfg, reps, n_devices=N_CORES):
    key = (cfg_key, reps, n_devices)
    if key not in _CACHE:
        _CACHE[key] = _build(cfg, reps=reps, n_devices=n_devices)
    return _CACHE[key]


def _make_in_maps(cfg, data, x, inputs, n_devices=N_CORES):
    ncls = cfg["n_classes"]
    W_bf = [np.asarray(inputs[k], np.float32).astype(ml_dtypes.bfloat16)
            for k in ("W1", "W2", "W3")]
    m = {
        "x": x.astype(np.float32),
        "S": data["S"],
        "idx0": data["idx0"],
        "idx1": data["idx1"],
        "deg_pt": data["deg_pt"],
        "W1": W_bf[0], "W2": W_bf[1], "W3": W_bf[2],
        "gamma": np.asarray(inputs["gamma"], np.float32).reshape(D, 1),
        "beta": np.asarray(inputs["beta"], np.float32).reshape(D, 1),
        "lin1w": np.asarray(inputs["lin1_w"],
                            np.float32).astype(ml_dtypes.bfloat16),
        "lin1b": np.asarray(inputs["lin1_b"], np.float32).reshape(D, 1),
        "lin2w": np.asarray(inputs["lin2_w"],
                            np.float32).astype(ml_dtypes.bfloat16),
        "lin2b": np.asarray(inputs["lin2_b"], np.float32).reshape(ncls, 1),
    }
    return [m for _ in range(n_devices)]


def kernel(x, edge_index, batch, W1, b1, W2, b2, W3, b3, gamma, beta,
           lin1_w, lin1_b, lin2_w, lin2_b, _reps=1, _ndev=1):
    x = np.asarray(x, np.float32)
    edge_index = np.asarray(edge_index)
    batch = np.asarray(batch)
    n_nodes, d = x.shape
    ncls = np.asarray(lin2_w).shape[1]
    assert d == D

    cfg, data = _prep(x, edge_index, batch, ncls)

    # NOTE: b1/b2/b3 cancel inside BatchNorm (mean subtraction) - unused.
    in_maps = _make_in_maps(cfg, data, x, {
        "W1": W1, "W2": W2, "W3": W3, "gamma": gamma, "beta": beta,
        "lin1_w": lin1_w, "lin1_b": lin1_b, "lin2_w": lin2_w,
        "lin2_b": lin2_b}, n_devices=_ndev)

    cfg_key = (n_nodes, edge_index.shape[1], ncls, cfg["NBLK"])
    nc = _get_built(cfg_key, cfg, _reps, _ndev)
    res = run_bass_kernel_spmd(nc, in_maps, core_ids=list(range(_ndev)))
    return np.asarray(res.results[0]["out"]).astype(np.float32)


# revision 18
# speedup vs baseline: 3.6561x; 1.8790x over previous
"""Trainium2 Bass kernel for nn_GCN1PoolNorm: 3-layer GCN + shared BatchNorm +
global max pool + MLP head.

Self-contained: takes FULL inputs, returns FULL output [N_GRAPHS, N_CLASSES].

v3 design — FULLY REPLICATED, ZERO COLLECTIVES:
On this runner every collective costs ~27-38 ms (software-emulated NRT), so
any sharded design is collective-bound. Instead every core runs the whole
graph; the host takes core 0's output. No cross-core traffic at all.

Per layer (full graph, N=50000 nodes, E=850k edges incl. self loops):
- Node features h_tilde = act * dis live in a local DRAM table [N, 128] bf16
  (cols 0:64 = payload, 64:128 = zero pad -> 256B rows for SWDGE dma_gather).
- Host sorts edges by (dst window of 128, src half, src), pads each
  (window, half) group to 128-edge blocks. Per chunk of WC windows:
  2 dma_gathers (int16 idx limit -> src < 32768 and rest) fetch h_tilde[src]
  rows edge-major; S one-hot blocks [128e, 128d] fp8 stream from DRAM.
- Aggregation per window: chained PE matmuls psum[128d, 64f] +=
  S_blk.T @ msgs_blk[:, 0:64]  (node-major).
- Downstream per 4-window group: U = ACT(psum) * dis (per-partition scale),
  PE transpose to feat-major, Z = W.T @ U, BN stats via ACT accum_out
  (locally -> global stats without any AllReduce), z spilled to DRAM.
- Act phase per group: z -> BN affine + relu -> (layers 1,2) transpose back,
  * dis, write table rows; (layer 3) pool graph segments on the fly.
- Head computed for all 64 graphs on every core.
"""
import numpy as np
import ml_dtypes

from concourse import bacc, mybir, tile
from concourse.bass_utils import run_bass_kernel_spmd
from concourse.masks import make_identity

f32 = mybir.dt.float32
bf16 = mybir.dt.bfloat16
fp8 = mybir.dt.float8e4
i16 = mybir.dt.int16

N_CORES = 8
P = 128          # partition / block / dst-window quantum
D = 64           # feature dim
HALF = 32768     # int16 gather index limit
TROW = 128       # table row width (64 feats + 64 zero pad) -> 256B rows
BN_EPS = 1e-5
WC = 8           # dst windows per stream chunk
GW = 4           # windows per downstream group (512 nodes)


# ---------------------------------------------------------------- host prep

def _prep(x, edge_index, batch, n_classes):
    n_nodes = x.shape[0]
    n_graphs = int(batch.max()) + 1
    ntile = (n_nodes + P - 1) // P              # dst windows
    tsz = [min(P, n_nodes - t * P) for t in range(ntile)]

    src = edge_index[0].astype(np.int64)
    dst = edge_index[1].astype(np.int64)
    deg = np.bincount(dst, minlength=n_nodes).astype(np.int64)

    # edge stream (incl. self loops), sorted by (window, half, src)
    s = np.concatenate([src, np.arange(n_nodes)])
    dl = np.concatenate([dst, np.arange(n_nodes)])
    t = dl // P
    h = (s >= HALF).astype(np.int64)
    col = dl % P
    order = np.lexsort((s, h, t))
    s, t, h, col = s[order], t[order], h[order], col[order]
    cnts = np.zeros((ntile, 2), np.int64)
    np.add.at(cnts, (t, h), 1)

    nb = (cnts + P - 1) // P                    # [ntile, 2] blocks per group
    NB0 = int(nb[:, 0].sum())
    NB1 = int(nb[:, 1].sum())
    NBLK = NB0 + NB1
    cum0 = np.concatenate([[0], np.cumsum(nb[:, 0])]).astype(int)
    cum1 = np.concatenate([[0], np.cumsum(nb[:, 1])]).astype(int)
    sb = np.concatenate([[0], np.cumsum(nb.sum(axis=1))]).astype(int)

    idx0 = np.zeros(NB0 * P, np.int16)
    idx1 = np.zeros(NB1 * P, np.int16)
    S = np.zeros((P, NBLK, P), ml_dtypes.float8_e4m3)
    off = 0
    for tt in range(ntile):
        for hh in (0, 1):
            n = int(cnts[tt, hh])
            e = slice(off, off + n)
            off += n
            q = np.arange(n)
            if hh == 0:
                idx0[cum0[tt] * P:cum0[tt] * P + n] = s[e]
                blk0 = sb[tt]
            else:
                idx1[cum1[tt] * P:cum1[tt] * P + n] = s[e] - HALF
                blk0 = sb[tt] + nb[tt, 0]
            S[q % P, blk0 + q // P, col[e]] = 1.0
    # wrap indices: idx g -> [g % 16, g // 16], replicated on 128 partitions
    idx0w = np.tile(idx0.reshape(-1, 16).T, (8, 1)).copy()
    idx1w = np.tile(idx1.reshape(-1, 16).T, (8, 1)).copy()

    # deg layouts (fp32)
    deg_pt = np.zeros((P, ntile), np.float32)
    degf = deg.astype(np.float32)
    for tt in range(ntile):
        deg_pt[:tsz[tt], tt] = degf[tt * P:tt * P + tsz[tt]]

    # pooling segments grouped by 512-node groups (for on-the-fly pooling)
    gb = np.searchsorted(batch, np.arange(n_graphs + 1))
    ngrp = (ntile + GW - 1) // GW
    pool_segs = []                              # (grp, s0, s1, g) rel to group
    for gr in range(ngrp):
        a, b = gr * GW * P, min((gr + 1) * GW * P, n_nodes)
        for g in range(n_graphs):
            s0, e0 = max(a, int(gb[g])), min(b, int(gb[g + 1]))
            if s0 < e0:
                pool_segs.append((gr, s0 - a, e0 - a, g))

    cfg = dict(n_nodes=n_nodes, ntile=ntile, tsz=tsz,
               nb=nb.tolist(), NB0=NB0, NB1=NB1, NBLK=NBLK,
               cum0=cum0.tolist(), cum1=cum1.tolist(), sb=sb.tolist(),
               pool_segs=pool_segs, ngrp=ngrp,
               n_classes=n_classes, n_graphs=n_graphs)
    data = dict(idx0=idx0w, idx1=idx1w, S=S, deg_pt=deg_pt)
    return cfg, data


# ---------------------------------------------------------------- device build

def _build(cfg, reps=1, n_devices=N_CORES):
    ntile, tsz = cfg["ntile"], cfg["tsz"]
    nb, NB0, NB1, NBLK = cfg["nb"], cfg["NB0"], cfg["NB1"], cfg["NBLK"]
    cum0, cum1, sb = cfg["cum0"], cfg["cum1"], cfg["sb"]
    ncls = cfg["n_classes"]
    n_nodes, n_graphs = cfg["n_nodes"], cfg["n_graphs"]
    ngrp = cfg["ngrp"]
    ntp = ntile * P
    RW = 512

    chunks = [(t0, min(t0 + WC, ntile)) for t0 in range(0, ntile, WC)]
    CB0 = max(cum0[t1] - cum0[t0] for t0, t1 in chunks)
    CB1 = max(cum1[t1] - cum1[t0] for t0, t1 in chunks)
    CBS = max(sb[t1] - sb[t0] for t0, t1 in chunks)

    nc = bacc.Bacc(trn_type="TRN2", target_bir_lowering=False, debug=False,
                   num_devices=n_devices, num_swdge_queues=4,
                   dynamic_dma_scratch_size=65536)

    x_in = nc.dram_tensor("x", [n_nodes, D], f32, kind="ExternalInput").ap()
    S_in = nc.dram_tensor("S", [P, NBLK, P], fp8, kind="ExternalInput").ap()
    idx0_in = nc.dram_tensor("idx0", [P, NB0 * 8], i16, kind="ExternalInput").ap()
    idx1_in = nc.dram_tensor("idx1", [P, NB1 * 8], i16, kind="ExternalInput").ap()
    deg_pt = nc.dram_tensor("deg_pt", [P, ntile], f32, kind="ExternalInput").ap()
    Ws = [nc.dram_tensor(f"W{i}", [D, D], bf16, kind="ExternalInput").ap()
          for i in (1, 2, 3)]
    gamma = nc.dram_tensor("gamma", [D, 1], f32, kind="ExternalInput").ap()
    beta = nc.dram_tensor("beta", [D, 1], f32, kind="ExternalInput").ap()
    lin1w = nc.dram_tensor("lin1w", [D, D], bf16, kind="ExternalInput").ap()
    lin1b = nc.dram_tensor("lin1b", [D, 1], f32, kind="ExternalInput").ap()
    lin2w = nc.dram_tensor("lin2w", [D, ncls], bf16, kind="ExternalInput").ap()
    lin2b = nc.dram_tensor("lin2b", [ncls, 1], f32, kind="ExternalInput").ap()
    out = nc.dram_tensor("out", [n_graphs, ncls], f32,
                         kind="ExternalOutput").ap()

    table = nc.dram_tensor("table", [ntp, TROW], bf16).ap()
    z_dram = nc.dram_tensor("z_dram", [D, ntp], bf16).ap()

    with tile.TileContext(nc) as tc:
        with (
            tc.tile_pool(name="const", bufs=1) as cpool,
            tc.tile_pool(name="sbuf_s", bufs=2) as spool,
            tc.tile_pool(name="msgs", bufs=2) as mpool,
            tc.tile_pool(name="idxp", bufs=2) as ipool,
            tc.tile_pool(name="work", bufs=3) as wpool,
            tc.tile_pool(name="ump", bufs=6) as umpool,
            tc.tile_pool(name="psacc", bufs=2, space="PSUM") as ps_acc,
            tc.tile_pool(name="pstru", bufs=2, space="PSUM") as ps_tru,
            tc.tile_pool(name="psz", bufs=2, space="PSUM") as ps_z,
        ):
            # ---- residents
            dis_pt = cpool.tile([P, ntile], f32)
            sums = cpool.tile([D, ngrp], f32)
            sums2 = cpool.tile([D, ngrp], f32)
            W_sb = [cpool.tile([D, D], bf16, tag=f"W{i}", name=f"W{i}_sb")
                    for i in range(3)]
            for i in range(3):
                nc.sync.dma_start(out=W_sb[i][:], in_=Ws[i][:])
            gamma_sb = cpool.tile([D, 1], f32, tag="gamma")
            beta_sb = cpool.tile([D, 1], f32, tag="beta")
            nc.sync.dma_start(out=gamma_sb[:], in_=gamma[:])
            nc.sync.dma_start(out=beta_sb[:], in_=beta[:])
            l1w_sb = cpool.tile([D, D], bf16, tag="l1w")
            l1b_sb = cpool.tile([D, 1], f32, tag="l1b")
            l2w_sb = cpool.tile([D, ncls], bf16, tag="l2w")
            l2b_sb = cpool.tile([ncls, 1], f32, tag="l2b")
            nc.sync.dma_start(out=l1w_sb[:], in_=lin1w[:])
            nc.sync.dma_start(out=l1b_sb[:], in_=lin1b[:])
            nc.sync.dma_start(out=l2w_sb[:], in_=lin2w[:])
            nc.sync.dma_start(out=l2b_sb[:], in_=lin2b[:])
            identP = cpool.tile([P, P], bf16, tag="identP")
            make_identity(nc, identP[:])
            emb = cpool.tile([D, n_graphs], f32, tag="emb")
            eps_sb = cpool.tile([D, 1], f32, tag="eps")
            nc.gpsimd.memset(eps_sb[:], BN_EPS)

            # one-time zero of table right half + pad rows
            zpad = cpool.tile([P, TROW], bf16, tag="zpad")
            nc.gpsimd.memset(zpad[:], 0.0)
            for tt in range(ntile):
                w = tsz[tt]
                nc.sync.dma_start(out=table[tt * P:tt * P + w, D:TROW],
                                  in_=zpad[:w, :D])
                if w < P:
                    nc.sync.dma_start(out=table[tt * P + w:(tt + 1) * P, :],
                                      in_=zpad[:P - w, :])

            # ---- dis (node-major per-partition layout only)
            dptf = wpool.tile([P, ntile], f32, tag="dptf")
            nc.sync.dma_start(out=dptf[:], in_=deg_pt[:])
            nc.scalar.activation(dis_pt[:], dptf[:],
                                 mybir.ActivationFunctionType.Sqrt, bias=1.0)
            nc.vector.reciprocal(dis_pt[:], dis_pt[:])

            # ---- table0 = bf16(x * dis)
            for tt in range(ntile):
                w = tsz[tt]
                xt = wpool.tile([P, D], f32, tag="xt")
                nc.sync.dma_start(out=xt[:w, :], in_=x_in[tt * P:tt * P + w, :])
                xb = wpool.tile([P, D], bf16, tag="xb")
                nc.scalar.activation(xb[:w, :], xt[:w, :],
                                     mybir.ActivationFunctionType.Copy,
                                     scale=dis_pt[:w, tt:tt + 1])
                nc.sync.dma_start(out=table[tt * P:tt * P + w, 0:D],
                                  in_=xb[:w, :])

            # ---- layers
            for rep in range(reps):
                for li in range(3):
                    last = (li == 2)
                    Wl = W_sb[li]
                    # aggregation + z, chunked
                    pend = []          # per-window node-major U psum tiles
                    gdone = 0

                    def flush_group(pend_tiles, gidx):
                        # pend_tiles: list of (t, w, um_tile)
                        ptru = ps_tru.tile([D, RW], bf16, tag="tru",
                                           space="PSUM")
                        gw = 0
                        for (tt_, w_, um_) in pend_tiles:
                            nc.tensor.transpose(ptru[:, gw:gw + w_],
                                                um_[:w_, :], identP[:w_, :w_])
                            gw += w_
                        ut = wpool.tile([D, RW], bf16, tag="ut")
                        nc.vector.tensor_copy(ut[:, :gw], ptru[:, :gw])
                        psz = ps_z.tile([D, RW], f32, tag="zt", space="PSUM")
                        nc.tensor.matmul(psz[:, :gw], lhsT=Wl[:],
                                         rhs=ut[:, :gw],
                                         start=True, stop=True)
                        zt = wpool.tile([D, RW], bf16, tag="ztile")
                        nc.scalar.activation(
                            zt[:, :gw], psz[:, :gw],
                            mybir.ActivationFunctionType.Copy,
                            accum_out=sums[:, gidx:gidx + 1])
                        sq = wpool.tile([D, RW], f32, tag="sq")
                        nc.scalar.activation(
                            sq[:, :gw], psz[:, :gw],
                            mybir.ActivationFunctionType.Square,
                            accum_out=sums2[:, gidx:gidx + 1])
                        g0 = pend_tiles[0][0] * P
                        nc.sync.dma_start(out=z_dram[:, g0:g0 + gw],
                                          in_=zt[:, :gw])

                    for (t0, t1) in chunks:
                        nb0c = cum0[t1] - cum0[t0]
                        nb1c = cum1[t1] - cum1[t0]
                        nbsc = sb[t1] - sb[t0]
                        S_t = spool.tile([P, CBS, P], fp8, tag="S")
                        nc.sync.dma_start(out=S_t[:, :nbsc, :],
                                          in_=S_in[:, sb[t0]:sb[t1], :])
                        m0 = mpool.tile([P, CB0, TROW], bf16, tag="m0")
                        if nb0c:
                            ix0 = ipool.tile([P, CB0 * 8], i16, tag="ix0")
                            nc.sync.dma_start(
                                out=ix0[:, :nb0c * 8],
                                in_=idx0_in[:, cum0[t0] * 8:cum0[t1] * 8])
                            na = (nb0c + 1) // 2
                            nc.gpsimd.dma_gather(
                                m0[:, :na, :], table[:HALF, :],
                                ix0[:, :na * 8],
                                na * P, na * P, TROW,
                                single_packet=False, queue_num=0)
                            if nb0c > na:
                                nc.gpsimd.dma_gather(
                                    m0[:, na:nb0c, :], table[:HALF, :],
                                    ix0[:, na * 8:nb0c * 8],
                                    (nb0c - na) * P, (nb0c - na) * P, TROW,
                                    single_packet=False, queue_num=2)
                        m1 = mpool.tile([P, CB1, TROW], bf16, tag="m1")
                        if nb1c:
                            ix1 = ipool.tile([P, CB1 * 8], i16, tag="ix1")
                            nc.sync.dma_start(
                                out=ix1[:, :nb1c * 8],
                                in_=idx1_in[:, cum1[t0] * 8:cum1[t1] * 8])
                            nb_ = (nb1c + 1) // 2
                            nc.gpsimd.dma_gather(
                                m1[:, :nb_, :], table[HALF:ntp, :],
                                ix1[:, :nb_ * 8],
                                nb_ * P, nb_ * P, TROW,
                                single_packet=False, queue_num=1)
                            if nb1c > nb_:
                                nc.gpsimd.dma_gather(
                                    m1[:, nb_:nb1c, :], table[HALF:ntp, :],
                                    ix1[:, nb_ * 8:nb1c * 8],
                                    (nb1c - nb_) * P, (nb1c - nb_) * P, TROW,
                                    single_packet=False, queue_num=3)
                        for tt in range(t0, t1):
                            w = tsz[tt]
                            nblocks = nb[tt][0] + nb[tt][1]
                            ps = ps_acc.tile([P, D], f32, tag="acc",
                                             space="PSUM")
                            for i in range(nblocks):
                                if i < nb[tt][0]:
                                    rhs = m0[:, cum0[tt] - cum0[t0] + i, 0:D]
                                else:
                                    rhs = m1[:, cum1[tt] - cum1[t0]
                                             + (i - nb[tt][0]), 0:D]
                                lhs = S_t[:, sb[tt] - sb[t0] + i, :]
                                nc.tensor.matmul(
                                    ps[:], lhsT=lhs, rhs=rhs,
                                    start=(i == 0), stop=(i == nblocks - 1))
                            # U node-major = psum * dis (per-partition scale)
                            um = umpool.tile([P, D], bf16, tag="um")
                            nc.scalar.activation(
                                um[:w, :], ps[:w, :],
                                mybir.ActivationFunctionType.Copy,
                                scale=dis_pt[:w, tt:tt + 1])
                            pend.append((tt, w, um))
                            if len(pend) == GW:
                                flush_group(pend, gdone)
                                pend = []
                                gdone += 1
                    if pend:
                        flush_group(pend, gdone)
                        pend = []
                        gdone += 1

                    # ---- global BN stats (local reduction — no collective)
                    st = wpool.tile([D, 2], f32, tag="st")
                    nc.vector.reduce_sum(st[:, 0:1], sums[:],
                                         axis=mybir.AxisListType.X)
                    nc.vector.reduce_sum(st[:, 1:2], sums2[:],
                                         axis=mybir.AxisListType.X)
                    mu = wpool.tile([D, 1], f32, tag="mu")
                    nc.scalar.activation(mu[:], st[:, 0:1],
                                         mybir.ActivationFunctionType.Copy,
                                         scale=1.0 / n_nodes)
                    va = wpool.tile([D, 1], f32, tag="va")
                    nc.scalar.activation(va[:], st[:, 1:2],
                                         mybir.ActivationFunctionType.Copy,
                                         scale=1.0 / n_nodes)
                    mu2 = wpool.tile([D, 1], f32, tag="mu2")
                    nc.vector.tensor_tensor(out=mu2[:], in0=mu[:], in1=mu[:],
                                            op=mybir.AluOpType.mult)
                    nc.vector.tensor_tensor(out=va[:], in0=va[:], in1=mu2[:],
                                            op=mybir.AluOpType.subtract)
                    nc.scalar.activation(va[:], va[:],
                                         mybir.ActivationFunctionType.Sqrt,
                                         bias=eps_sb[:])
                    nc.vector.reciprocal(va[:], va[:])
                    saff = wpool.tile([D, 1], f32, tag="saff")
                    nc.vector.tensor_tensor(out=saff[:], in0=gamma_sb[:],
                                            in1=va[:], op=mybir.AluOpType.mult)
                    tsh_ = wpool.tile([D, 1], f32, tag="tsh")
                    nc.vector.tensor_tensor(out=tsh_[:], in0=mu[:], in1=saff[:],
                                            op=mybir.AluOpType.mult)
                    nc.vector.tensor_tensor(out=tsh_[:], in0=beta_sb[:],
                                            in1=tsh_[:],
                                            op=mybir.AluOpType.subtract)

                    # ---- activation phase per group
                    first_seen = set()
                    for gr in range(ngrp):
                        a = gr * GW * P
                        b = min((gr + 1) * GW * P, n_nodes)
                        gw = b - a
                        zt2 = wpool.tile([D, RW], bf16, tag="zt2")
                        nc.sync.dma_start(out=zt2[:, :gw],
                                          in_=z_dram[:, a:a + gw])
                        at = wpool.tile([D, RW], bf16, tag="at")
                        nc.scalar.activation(at[:, :gw], zt2[:, :gw],
                                             mybir.ActivationFunctionType.Relu,
                                             bias=tsh_[:], scale=saff[:])
                        if not last:
                            for tt in range(gr * GW, min((gr + 1) * GW, ntile)):
                                w = tsz[tt]
                                o = tt * P - a
                                ptr = ps_tru.tile([P, D], bf16,
                                                  tag="trp", space="PSUM")
                                nc.tensor.transpose(ptr[:w, :],
                                                    at[:, o:o + w],
                                                    identP[:D, :D])
                                wr = wpool.tile([P, D], bf16, tag="wr")
                                nc.scalar.activation(
                                    wr[:w, :], ptr[:w, :],
                                    mybir.ActivationFunctionType.Copy,
                                    scale=dis_pt[:w, tt:tt + 1])
                                nc.sync.dma_start(
                                    out=table[tt * P:tt * P + w, 0:D],
                                    in_=wr[:w, :])
                        elif rep == reps - 1:
                            # pool graph segments on the fly
                            for (gr_, s0, s1, g) in cfg["pool_segs"]:
                                if gr_ != gr:
                                    continue
                                tmp = wpool.tile([D, 1], f32, tag="ptmp")
                                nc.vector.reduce_max(
                                    tmp[:], at[:, s0:s1],
                                    axis=mybir.AxisListType.X)
                                if g not in first_seen:
                                    first_seen.add(g)
                                    nc.vector.tensor_copy(emb[:, g:g + 1],
                                                          tmp[:])
                                else:
                                    nc.vector.tensor_tensor(
                                        out=emb[:, g:g + 1],
                                        in0=emb[:, g:g + 1], in1=tmp[:],
                                        op=mybir.AluOpType.max)

            # ---- head (all graphs, every core)
            emb_bf = wpool.tile([D, n_graphs], bf16, tag="embbf")
            nc.vector.tensor_copy(emb_bf[:], emb[:])
            ph_full = ps_z.tile([D, RW], f32, tag="zt", space="PSUM")
            ph = ph_full[:, :n_graphs]
            nc.tensor.matmul(ph, lhsT=l1w_sb[:], rhs=emb_bf[:],
                             start=True, stop=True)
            h1 = wpool.tile([D, n_graphs], bf16, tag="h1")
            nc.scalar.activation(h1[:], ph,
                                 mybir.ActivationFunctionType.Relu,
                                 bias=l1b_sb[:])
            po_full = ps_z.tile([D, RW], f32, tag="zt", space="PSUM")
            po = po_full[:ncls, :n_graphs]
            nc.tensor.matmul(po, lhsT=l2w_sb[:], rhs=h1[:],
                             start=True, stop=True)
            osb = wpool.tile([ncls, n_graphs], f32, tag="osb")
            nc.scalar.activation(osb[:], po,
                                 mybir.ActivationFunctionType.Identity,
                                 bias=l2b_sb[:])
            nc.sync.dma_start(out=out[:, :].rearrange("g c -> c g"), in_=osb[:])

    nc.compile()
    return nc


# ---------------------------------------------------------------- entry point

_CACHE = {}


def _get_built(cfg_key, cfg, reps, n_devices=N_CORES):
    key = (cfg_key, reps, n_devices)
    if key not in _CACHE:
        _CACHE[key] = _build(cfg, reps=reps, n_devices=n_devices)
    return _CACHE[key]


def _make_in_maps(cfg, data, x, inputs, n_devices=N_CORES):
    ncls = cfg["n_classes"]
    W_bf = [np.asarray(inputs[k], np.float32).astype(ml_dtypes.bfloat16)
            for k in ("W1", "W2", "W3")]
    m = {
        "x": x.astype(np.float32),
        "S": data["S"],
        "idx0": data["idx0"],
        "idx1": data["idx1"],
        "deg_pt": data["deg_pt"],
        "W1": W_bf[0], "W2": W_bf[1], "W3": W_bf[2],
        "gamma": np.asarray(inputs["gamma"], np.float32).reshape(D, 1),
        "beta": np.asarray(inputs["beta"], np.float32).reshape(D, 1),
        "lin1w": np.asarray(inputs["lin1_w"],
                            np.float32).astype(ml_dtypes.bfloat16),
        "lin1b": np.asarray(inputs["lin1_b"], np.float32).reshape(D, 1),
        "lin2w": np.asarray(inputs["lin2_w"],
                            np.float32).astype(ml_dtypes.bfloat16),
        "lin2b": np.asarray(inputs["lin2_b"], np.float32).reshape(ncls, 1),
    }
    return [m for _ in range(n_devices)]


def kernel(x, edge_index, batch, W1, b1, W2, b2, W3, b3, gamma, beta,
           lin1_w, lin1_b, lin2_w, lin2_b, _reps=1, _ndev=1):
    x = np.asarray(x, np.float32)
    edge_index = np.asarray(edge_index)
    batch = np.asarray(batch)
    n_nodes, d = x.shape
    ncls = np.asarray(lin2_w).shape[1]
    assert d == D

    cfg, data = _prep(x, edge_index, batch, ncls)

    # NOTE: b1/b2/b3 cancel inside BatchNorm (mean subtraction) - unused.
    in_maps = _make_in_maps(cfg, data, x, {
        "W1": W1, "W2": W2, "W3": W3, "gamma": gamma, "beta": beta,
        "lin1_w": lin1_w, "lin1_b": lin1_b, "lin2_w": lin2_w,
        "lin2_b": lin2_b}, n_devices=_ndev)

    cfg_key = (n_nodes, edge_index.shape[1], ncls, cfg["NBLK"])
    nc = _get_built(cfg_key, cfg, _reps, _ndev)
    res = run_bass_kernel_spmd(nc, in_maps, core_ids=list(range(_ndev)))
    return np.asarray(res.results[0]["out"]).astype(np.float32)
